# revision 26
# baseline (speedup 1.0000x reference)
"""GQA attention layer (B=1, S=2048, D=4096, H=32, KV=8, HD=128) on 8 TRN2
NeuronCores. Tensor-parallel over the 8 KV-head groups; per-head AllGather of
the attention outputs (overlapped with attention compute), then each core
computes one column shard of the output projection (no AllReduce needed).

Datapath is bf16 into fp32 PSUM accumulation (x/wq/wkv/q/k/v/P/O/wo in bf16;
scores, softmax sums and the output projection accumulate in fp32). Attention
is computed in transposed orientation (scores^T = kT-slices as lhsT against
qT) so softmax sums become TensorEngine matmuls and no P-transposes are
needed. Exp runs on paired score tiles (1024-wide) to amortize ACT overhead.
The output projection accumulates per-head partials into an SBUF fp32
accumulator so each head's matmuls start as soon as its AllGather lands.
RoPE pairs are de-interleaved by permuting wq/wk columns host-side; all DRAM
operands are host-pretiled for contiguous DMA.
"""
import numpy as np
import ml_dtypes

S = 2048
D = 4096
HD = 128
QH = 4            # q heads per core
NCORES = 8
ST = S // 128     # 16 s-tiles
DK = D // 128     # 32 contraction tiles
G = 4             # q groups per head
SG = S // G       # 512 q positions per group
SCALE = 1.0 / np.sqrt(128.0)

_CACHE = {}


def _build():
    import concourse.mybir as mybir
    import concourse.tile as tile
    from concourse import bacc

    f32, f32r = mybir.dt.float32, mybir.dt.float32r
    bf16 = mybir.dt.bfloat16
    nc = bacc.Bacc("TRN2", target_bir_lowering=False, debug=False,
                   num_devices=NCORES)

    xt = nc.dram_tensor("xt", [ST, 128, DK, 128], bf16, kind="ExternalInput").ap()
    wq = nc.dram_tensor("wq", [128, DK, QH * HD], bf16, kind="ExternalInput").ap()
    wkv = nc.dram_tensor("wkv", [128, DK, 2 * HD], bf16, kind="ExternalInput").ap()
    wo = nc.dram_tensor("wo", [128, DK, 512], bf16, kind="ExternalInput").ap()
    cos = nc.dram_tensor("cos", [128, ST, 64], f32, kind="ExternalInput").ap()
    sin = nc.dram_tensor("sin", [128, ST, 64], f32, kind="ExternalInput").ap()
    madd = nc.dram_tensor("madd", [128, 128], f32, kind="ExternalInput").ap()
    onesc = nc.dram_tensor("onesc", [128, 1], f32r, kind="ExternalInput").ap()
    onesr = nc.dram_tensor("onesr", [1, 128], f32r, kind="ExternalInput").ap()
    ident = nc.dram_tensor("ident", [128, 128], bf16, kind="ExternalInput").ap()
    yt = nc.dram_tensor("yt", [512, S], f32, kind="ExternalOutput").ap()

    with tile.TileContext(nc) as tc:
        with (
            tc.tile_pool(name="const", bufs=1) as constp,
            tc.tile_pool(name="resid", bufs=1) as resid,
            tc.tile_pool(name="dram", bufs=1, space="DRAM") as dram,
        ):
            cos_sb = constp.tile([128, ST, 64], f32)
            sin_sb = constp.tile([128, ST, 64], f32)
            madd_sb = constp.tile([128, 128], f32)
            onesc_sb = constp.tile([128, 1], f32r)
            onesr_sb = constp.tile([1, 128], f32r)
            ident_sb = constp.tile([128, 128], bf16)
            nc.gpsimd.dma_start(cos_sb[:], cos)
            nc.gpsimd.dma_start(sin_sb[:], sin)
            nc.gpsimd.dma_start(madd_sb[:], madd)
            nc.gpsimd.dma_start(onesc_sb[:], onesc)
            nc.gpsimd.dma_start(onesr_sb[:], onesr)
            nc.gpsimd.dma_start(ident_sb[:], ident)

            # residents across phases
            kt_sb = resid.tile([128, S], bf16)           # roped K^T  [d, s]
            v_sb = resid.tile([128, ST, HD], bf16)       # natural V  [s-in-tile, st, d]

            qt_spill = dram.tile([QH * 128, S], bf16)    # roped Q^T per head
            cc_in = [dram.tile([128, S], bf16, name=f"cc_in{h}") for h in range(QH)]
            cc_out = [dram.tile([NCORES * 128, S], bf16, addr_space="Shared",
                                name=f"cc_out{h}") for h in range(QH)]

            # ---------------- Phase 1: QKV projections + RoPE + transposes
            with (
                tc.tile_pool(name="wpool", bufs=1) as wpool,
                tc.tile_pool(name="xpool", bufs=3) as xpool,
                tc.tile_pool(name="p1tmp", bufs=3) as p1tmp,
                tc.tile_pool(name="p1psum", bufs=2, space="PSUM") as p1psum,
            ):
                wq_sb = wpool.tile([128, DK, QH * HD], bf16)
                wkv_sb = wpool.tile([128, DK, 2 * HD], bf16)

                def emit_transposes(qnat, knat, ssl):
                    # deferred one s-tile so PE never waits on the DVE rope
                    qts = p1tmp.tile([128, QH, 128], bf16, name="qts")
                    for h in range(QH):
                        qtr = p1psum.tile([128, 128], bf16, name="qtr")
                        nc.tensor.transpose(qtr[:], qnat[:, h, :], ident_sb[:])
                        nc.vector.tensor_copy(qts[:, h, :], qtr[:])
                    nc.sync.dma_start(
                        qt_spill.rearrange("(h p) s -> p h s", p=128)[:, :, ssl],
                        qts[:])
                    ktr = p1psum.tile([128, 128], bf16, name="ktr")
                    nc.tensor.transpose(ktr[:], knat[:], ident_sb[:])
                    nc.vector.tensor_copy(kt_sb[:, ssl], ktr[:])

                pending = None
                for st in range(ST):
                    ssl = slice(st * 128, (st + 1) * 128)
                    xt_st = xpool.tile([128, DK, 128], bf16, name="xt_st")
                    if st == 0:
                        # interleave x/weight chunks so the first matmuls
                        # unblock as soon as the leading chunks land
                        for c in range(8):
                            ksl = slice(c * 4, (c + 1) * 4)
                            nc.sync.dma_start(xt_st[:, ksl, :], xt[st][:, ksl, :])
                            nc.sync.dma_start(wq_sb[:, ksl, :], wq[:, ksl, :])
                            nc.sync.dma_start(wkv_sb[:, ksl, :], wkv[:, ksl, :])
                    else:
                        nc.sync.dma_start(xt_st[:], xt[st])

                    q_ps = p1psum.tile([128, QH * HD], f32, name="q_ps")
                    kv_ps = p1psum.tile([128, 2 * HD], f32, name="kv_ps")
                    for kt in range(DK):
                        nc.tensor.matmul(q_ps[:], lhsT=xt_st[:, kt], rhs=wq_sb[:, kt],
                                         start=(kt == 0), stop=(kt == DK - 1))
                        nc.tensor.matmul(kv_ps[:], lhsT=xt_st[:, kt], rhs=wkv_sb[:, kt],
                                         start=(kt == 0), stop=(kt == DK - 1))
                    if pending is not None:
                        emit_transposes(*pending)

                    # RoPE on q (4 heads batched) during PSUM eviction.
                    qp = q_ps[:].rearrange("p (h d) -> p h d", h=QH)
                    qa, qb = qp[:, :, 0:64], qp[:, :, 64:128]
                    cbc = cos_sb[:, st:st + 1, :].to_broadcast([128, QH, 64])
                    sbc = sin_sb[:, st:st + 1, :].to_broadcast([128, QH, 64])
                    t1 = p1tmp.tile([128, QH, 64], f32, name="t1")
                    t2 = p1tmp.tile([128, QH, 64], f32, name="t2")
                    qnat = p1tmp.tile([128, QH, HD], bf16, name="qnat")
                    na, nb = qnat[:, :, 0:64], qnat[:, :, 64:128]
                    nc.vector.tensor_tensor(t1[:], qa, sbc, mybir.AluOpType.mult)
                    nc.vector.tensor_tensor(t2[:], qb, sbc, mybir.AluOpType.mult)
                    nc.vector.tensor_tensor(na, qa, cbc, mybir.AluOpType.mult)
                    nc.vector.tensor_tensor(nb, qb, cbc, mybir.AluOpType.mult)
                    nc.vector.tensor_tensor(na, na, t2[:], mybir.AluOpType.subtract)
                    nc.vector.tensor_tensor(nb, nb, t1[:], mybir.AluOpType.add)

                    # RoPE on k
                    ka, kb = kv_ps[:, 0:64], kv_ps[:, 64:128]
                    cb1 = cos_sb[:, st, :]
                    sb1 = sin_sb[:, st, :]
                    kt1 = p1tmp.tile([128, 64], f32, name="kt1")
                    kt2 = p1tmp.tile([128, 64], f32, name="kt2")
                    knat = p1tmp.tile([128, HD], bf16, name="knat")
                    kna, knb = knat[:, 0:64], knat[:, 64:128]
                    nc.vector.tensor_tensor(kt1[:], ka, sb1, mybir.AluOpType.mult)
                    nc.vector.tensor_tensor(kt2[:], kb, sb1, mybir.AluOpType.mult)
                    nc.vector.tensor_tensor(kna, ka, cb1, mybir.AluOpType.mult)
                    nc.vector.tensor_tensor(knb, kb, cb1, mybir.AluOpType.mult)
                    nc.vector.tensor_tensor(kna, kna, kt2[:], mybir.AluOpType.subtract)
                    nc.vector.tensor_tensor(knb, knb, kt1[:], mybir.AluOpType.add)

                    # V natural, straight copy
                    nc.vector.tensor_copy(v_sb[:, st, :], kv_ps[:, HD:2 * HD])

                    pending = (qnat, knat, ssl)
                emit_transposes(*pending)

            # pools that live through phases 2 and 3
            with (
                tc.tile_pool(name="wopool", bufs=1) as wopool,
                tc.tile_pool(name="ohpool", bufs=2) as ohpool,
                tc.tile_pool(name="yaccp", bufs=1) as yaccp,
            ):
                wo_sb = wopool.tile([128, DK, 512], bf16)
                nc.gpsimd.dma_start(wo_sb[:], wo)
                y_acc = yaccp.tile([128, 4, 4, SG], f32)
                oh_tiles = []

                # ---------------- Phase 2: attention per (head, group)
                with (
                    tc.tile_pool(name="p2tmp", bufs=3) as p2tmp,
                    tc.tile_pool(name="p2lb", bufs=2) as p2lb,
                    tc.tile_pool(name="p2psum", bufs=2, space="PSUM") as p2psum,
                    tc.tile_pool(name="p2opsum", bufs=2, space="PSUM") as p2opsum,
                ):
                    def finalize(fin):
                        # normalization chain for a finished group, emitted
                        # after the NEXT group's score prologue so the lb
                        # matmul never heads the PE queue while the DVE
                        # reciprocal chain is still in flight
                        fh, fgsl, fot, facc, last = fin
                        # single partition-sum matmul over the DVE-accumulated
                        # row partials (replaces one l-matmul per k-tile)
                        l_ps = p2opsum.tile([1, SG], f32, name="l_ps",
                                            tag="lnorm", bufs=1)
                        nc.tensor.matmul(l_ps[:], lhsT=onesc_sb[:],
                                         rhs=facc[:], start=True, stop=True)
                        linv_f = p2lb.tile([1, SG], f32, name="linv_f")
                        nc.vector.reciprocal_approx_fast(linv_f[:], l_ps[:])
                        linv_r = p2lb.tile([1, SG], f32r, name="linv_r")
                        nc.vector.tensor_copy(linv_r[:], linv_f[:])
                        lb_ps = p2opsum.tile([128, SG], f32, name="lb_ps",
                                             tag="lnorm", bufs=1)
                        nc.tensor.matmul(lb_ps[:], lhsT=onesr_sb[:],
                                         rhs=linv_r[:], start=True, stop=True)
                        lb_sb = p2lb.tile([128, SG], f32, name="lb_sb")
                        nc.vector.tensor_copy(lb_sb[:], lb_ps[:])
                        on_sb = p2tmp.tile([128, SG], bf16, name="on_sb")
                        nc.vector.tensor_tensor(on_sb[:], fot[:], lb_sb[:],
                                                mybir.AluOpType.mult)
                        nc.sync.dma_start(cc_in[fh][:, fgsl], on_sb[:])
                        if last:
                            nc.gpsimd.collective_compute(
                                "AllGather", mybir.AluOpType.bypass,
                                ins=[cc_in[fh].opt()], outs=[cc_out[fh].opt()],
                                replica_groups=[list(range(NCORES))],
                            )
                            # stage this head's gathered O into SBUF right
                            # away so the output projection can start the
                            # moment the PE queue drains phase 2. gpsimd
                            # queue: idle after startup, so the AG-wait here
                            # cannot stall the sync queue's qt_g prefetches
                            oh = ohpool.tile([128, NCORES, S], bf16, name="oh",
                                             tag="oh")
                            nc.gpsimd.dma_start(
                                oh[:],
                                cc_out[fh].rearrange("(r p) s -> p r s", p=128))
                            oh_tiles.append(oh)

                    pending_fin = None
                    for h in range(QH):
                        for g in range(G):
                            gsl = slice(g * SG, (g + 1) * SG)
                            qt_g = p2tmp.tile([128, SG], bf16, name="qt_g", bufs=4)
                            nc.sync.dma_start(
                                qt_g[:], qt_spill[h * 128:(h + 1) * 128, gsl])

                            nk = 4 * g + 4
                            DEPTH = 5
                            st_tiles = {}

                            def off_of(j):
                                # fully-masked q columns skipped on diagonal tiles
                                return max(0, (j - 4 * g)) * 128

                            def do_st(j, qt_g=qt_g, st_tiles=st_tiles,
                                      off_of=off_of):
                                off = off_of(j)
                                stp = p2psum.tile([128, SG], f32, name="st_ps",
                                                  tag="st_ps", bufs=DEPTH)
                                nc.tensor.matmul(
                                    stp[:, off:], lhsT=kt_sb[:, j * 128:(j + 1) * 128],
                                    rhs=qt_g[:, off:], start=True, stop=True)
                                if j >= 4 * g:
                                    # additive causal mask applied in PSUM,
                                    # ahead of the exp -> AV critical path
                                    nc.vector.tensor_tensor(
                                        stp[:, off:off + 128],
                                        stp[:, off:off + 128], madd_sb[:],
                                        mybir.AluOpType.add)
                                st_tiles[j] = stp

                            for j in range(min(DEPTH, nk)):
                                do_st(j)
                            if pending_fin is not None:
                                finalize(pending_fin)

                            ot_ps = p2opsum.tile([128, SG], f32, name="ot_ps", bufs=2)
                            l_acc = p2lb.tile([128, SG], f32r, name="l_acc",
                                              bufs=2)
                            for j in range(nk):
                                off = off_of(j)
                                st_ps = st_tiles.pop(j)
                                put = p2tmp.tile([128, SG], bf16, name="put", bufs=6)
                                nc.scalar.activation(put[:, off:], st_ps[:, off:],
                                                     mybir.ActivationFunctionType.Exp,
                                                     scale=SCALE)
                                nc.tensor.matmul(ot_ps[:, off:], lhsT=v_sb[:, j, :],
                                                 rhs=put[:, off:],
                                                 start=(j == 0), stop=(j == nk - 1),
                                                 skip_group_check=True)
                                # softmax-sum partials accumulate on the idle
                                # DVE instead of one PE matmul per k-tile
                                if j == 0:
                                    nc.vector.tensor_copy(l_acc[:], put[:])
                                else:
                                    nc.vector.tensor_tensor(
                                        l_acc[:, off:], l_acc[:, off:],
                                        put[:, off:], mybir.AluOpType.add)
                                if j + DEPTH < nk:
                                    do_st(j + DEPTH)

                            pending_fin = (h, gsl, ot_ps, l_acc, g == G - 1)
                    finalize(pending_fin)

                # ---------------- Phase 3: yT += wo_h^T @ O_h^T per head
                with (
                    tc.tile_pool(name="p3out", bufs=3) as p3out,
                    tc.tile_pool(name="p3psum", bufs=2, space="PSUM") as p3psum,
                ):
                    for h in range(QH):
                        oh = oh_tiles[h]
                        for dt in range(4):
                            y_ps = p3psum.tile([128, 4, SG], f32, name="y_ps",
                                               bufs=2)
                            for r in range(NCORES):
                                kt2 = h * NCORES + r
                                for sq in range(4):
                                    nc.tensor.matmul(
                                        y_ps[:, sq, :],
                                        lhsT=wo_sb[:, kt2,
                                                   dt * 128:(dt + 1) * 128],
                                        rhs=oh[:, r, sq * SG:(sq + 1) * SG],
                                        start=(r == 0), stop=(r == NCORES - 1))
                            for sq in range(4):
                                if h == 0:
                                    nc.vector.tensor_copy(
                                        y_acc[:, dt, sq, :], y_ps[:, sq, :])
                                elif h < QH - 1:
                                    nc.vector.tensor_tensor(
                                        y_acc[:, dt, sq, :],
                                        y_acc[:, dt, sq, :], y_ps[:, sq, :],
                                        mybir.AluOpType.add)
                                else:
                                    y_fin = p3out.tile([128, SG], f32,
                                                       name="y_fin")
                                    nc.vector.tensor_tensor(
                                        y_fin[:], y_acc[:, dt, sq, :],
                                        y_ps[:, sq, :], mybir.AluOpType.add)
                                    nc.sync.dma_start(
                                        yt[dt * 128:(dt + 1) * 128,
                                           sq * SG:(sq + 1) * SG], y_fin[:])
    nc.compile()
    return nc


def _host_prep(inputs):
    bf = ml_dtypes.bfloat16
    x = np.asarray(inputs["x"], dtype=np.float32)
    wq = np.asarray(inputs["wq"], dtype=np.float32)
    wk = np.asarray(inputs["wk"], dtype=np.float32)
    wv = np.asarray(inputs["wv"], dtype=np.float32)
    wo = np.asarray(inputs["wo"], dtype=np.float32)
    cos = np.asarray(inputs["freqs_cos"], dtype=np.float32)
    sin = np.asarray(inputs["freqs_sin"], dtype=np.float32)
    mask = np.asarray(inputs["mask"], dtype=np.float32)

    # xt[st, p, kt, s] = x[128*st + s, 128*kt + p]
    xts = np.ascontiguousarray(
        x.reshape(ST, 128, DK, 128).transpose(0, 3, 2, 1)).astype(bf)

    # de-interleave RoPE pairs within each head: evens then odds
    perm = np.concatenate([np.arange(0, HD, 2), np.arange(1, HD, 2)])

    cos_t = np.ascontiguousarray(cos.reshape(ST, 128, 64).transpose(1, 0, 2))
    sin_t = np.ascontiguousarray(sin.reshape(ST, 128, 64).transpose(1, 0, 2))

    # additive causal mask for a diagonal 128x128 tile [k-in-tile, q-in-tile],
    # derived from the actual mask input (g-independent for causal)
    madd_t = np.ascontiguousarray(
        np.where(mask[0:128, 0:128].T == 0.0, 0.0, -30000.0)).astype(np.float32)

    def ktile(w):  # [D, m] -> [128, DK, m]
        return np.ascontiguousarray(
            w.reshape(DK, 128, w.shape[1]).transpose(1, 0, 2))

    in_maps = []
    for c in range(NCORES):
        wq_c = wq[:, 512 * c:512 * (c + 1)].reshape(D, QH, HD)[:, :, perm]
        wq_c = wq_c.reshape(D, QH * HD)
        wk_c = wk[:, 128 * c:128 * (c + 1)][:, perm]
        wv_c = wv[:, 128 * c:128 * (c + 1)]
        wkv_c = np.concatenate([wk_c, wv_c], axis=1)
        # wo rows reordered to (head, rank, d) to match per-head AllGather
        wo_c = wo[:, 512 * c:512 * (c + 1)]
        wo_c = wo_c.reshape(NCORES, QH, 128, 512).transpose(1, 0, 2, 3)
        wo_c = wo_c.reshape(D, 512)
        in_maps.append({
            "xt": xts,
            "wq": ktile(wq_c).astype(bf),
            "wkv": ktile(wkv_c).astype(bf),
            "wo": ktile(wo_c).astype(bf),
            "cos": cos_t,
            "sin": sin_t,
            "madd": madd_t,
            "onesc": np.ones((128, 1), dtype=np.float32),
            "onesr": np.ones((1, 128), dtype=np.float32),
            "ident": np.eye(128, dtype=bf),
        })
    return in_maps


def _run(inputs, trace=False, tmpdir=None):
    from concourse import bass_utils
    if "nc" not in _CACHE:
        _CACHE["nc"] = _build()
    nc = _CACHE["nc"]
    in_maps = _host_prep(inputs)
    res = bass_utils.run_bass_kernel_spmd(
        nc, in_maps, core_ids=list(range(NCORES)), trace=trace, tmpdir=tmpdir)
    yts = [res.results[c]["yt"] for c in range(NCORES)]
    y = np.concatenate([t.T for t in yts], axis=1).astype(np.float32)
    return y.reshape(1, S, D), res


def kernel(**inputs):
    y, _ = _run(inputs, trace=False)
    return y


# revision 28
# speedup vs baseline: 1.0156x; 1.0156x over previous
"""GQA attention layer (B=1, S=2048, D=4096, H=32, KV=8, HD=128) on 8 TRN2
NeuronCores. Tensor-parallel over the 8 KV-head groups; per-head AllGather of
the attention outputs (overlapped with attention compute), then each core
computes one column shard of the output projection (no AllReduce needed).

Datapath is bf16 into fp32 PSUM accumulation (x/wq/wkv/q/k/v/P/O/wo in bf16;
scores, softmax sums and the output projection accumulate in fp32). Attention
is computed in transposed orientation (scores^T = kT-slices as lhsT against
qT) so softmax sums become TensorEngine matmuls and no P-transposes are
needed. Exp runs on paired score tiles (1024-wide) to amortize ACT overhead.
The output projection accumulates per-head partials into an SBUF fp32
accumulator so each head's matmuls start as soon as its AllGather lands.
RoPE pairs are de-interleaved by permuting wq/wk columns host-side; all DRAM
operands are host-pretiled for contiguous DMA.
"""
import numpy as np
import ml_dtypes

S = 2048
D = 4096
HD = 128
QH = 4            # q heads per core
NCORES = 8
ST = S // 128     # 16 s-tiles
DK = D // 128     # 32 contraction tiles
G = 4             # q groups per head
SG = S // G       # 512 q positions per group
SCALE = 1.0 / np.sqrt(128.0)

_CACHE = {}


def _build():
    import concourse.mybir as mybir
    import concourse.tile as tile
    from concourse import bacc

    f32, f32r = mybir.dt.float32, mybir.dt.float32r
    bf16 = mybir.dt.bfloat16
    nc = bacc.Bacc("TRN2", target_bir_lowering=False, debug=False,
                   num_devices=NCORES)

    xt = nc.dram_tensor("xt", [ST, 128, DK, 128], bf16, kind="ExternalInput").ap()
    wq = nc.dram_tensor("wq", [128, DK, QH * HD], bf16, kind="ExternalInput").ap()
    wkv = nc.dram_tensor("wkv", [128, DK, 2 * HD], bf16, kind="ExternalInput").ap()
    wo = nc.dram_tensor("wo", [128, DK, 512], bf16, kind="ExternalInput").ap()
    cos = nc.dram_tensor("cos", [128, ST, 64], f32, kind="ExternalInput").ap()
    sin = nc.dram_tensor("sin", [128, ST, 64], f32, kind="ExternalInput").ap()
    madd = nc.dram_tensor("madd", [128, 128], f32, kind="ExternalInput").ap()
    onesc = nc.dram_tensor("onesc", [128, 1], bf16, kind="ExternalInput").ap()
    onesr = nc.dram_tensor("onesr", [1, 128], f32r, kind="ExternalInput").ap()
    ident = nc.dram_tensor("ident", [128, 128], bf16, kind="ExternalInput").ap()
    yt = nc.dram_tensor("yt", [512, S], f32, kind="ExternalOutput").ap()

    with tile.TileContext(nc) as tc:
        with (
            tc.tile_pool(name="const", bufs=1) as constp,
            tc.tile_pool(name="resid", bufs=1) as resid,
            tc.tile_pool(name="dram", bufs=1, space="DRAM") as dram,
        ):
            cos_sb = constp.tile([128, ST, 64], f32)
            sin_sb = constp.tile([128, ST, 64], f32)
            madd_sb = constp.tile([128, 128], f32)
            onesc_sb = constp.tile([128, 1], bf16)
            onesr_sb = constp.tile([1, 128], f32r)
            ident_sb = constp.tile([128, 128], bf16)
            nc.gpsimd.dma_start(cos_sb[:], cos)
            nc.gpsimd.dma_start(sin_sb[:], sin)
            nc.gpsimd.dma_start(madd_sb[:], madd)
            nc.gpsimd.dma_start(onesc_sb[:], onesc)
            nc.gpsimd.dma_start(onesr_sb[:], onesr)
            nc.gpsimd.dma_start(ident_sb[:], ident)

            # residents across phases
            kt_sb = resid.tile([128, S], bf16)           # roped K^T  [d, s]
            v_sb = resid.tile([128, ST, HD], bf16)       # natural V  [s-in-tile, st, d]

            qt_spill = dram.tile([QH * 128, S], bf16)    # roped Q^T per head
            cc_in = [dram.tile([128, S], bf16, name=f"cc_in{h}") for h in range(QH)]
            cc_out = [dram.tile([NCORES * 128, S], bf16, addr_space="Shared",
                                name=f"cc_out{h}") for h in range(QH)]

            # ---------------- Phase 1: QKV projections + RoPE + transposes
            with (
                tc.tile_pool(name="wpool", bufs=1) as wpool,
                tc.tile_pool(name="xpool", bufs=3) as xpool,
                tc.tile_pool(name="p1tmp", bufs=3) as p1tmp,
                tc.tile_pool(name="p1psum", bufs=2, space="PSUM") as p1psum,
            ):
                wq_sb = wpool.tile([128, DK, QH * HD], bf16)
                wkv_sb = wpool.tile([128, DK, 2 * HD], bf16)

                def emit_transposes(qnat, knat, ssl):
                    # deferred one s-tile so PE never waits on the DVE rope
                    qts = p1tmp.tile([128, QH, 128], bf16, name="qts")
                    for h in range(QH):
                        qtr = p1psum.tile([128, 128], bf16, name="qtr")
                        nc.tensor.transpose(qtr[:], qnat[:, h, :], ident_sb[:])
                        nc.vector.tensor_copy(qts[:, h, :], qtr[:])
                    nc.sync.dma_start(
                        qt_spill.rearrange("(h p) s -> p h s", p=128)[:, :, ssl],
                        qts[:])
                    ktr = p1psum.tile([128, 128], bf16, name="ktr")
                    nc.tensor.transpose(ktr[:], knat[:], ident_sb[:])
                    nc.vector.tensor_copy(kt_sb[:, ssl], ktr[:])

                pending = None
                for st in range(ST):
                    ssl = slice(st * 128, (st + 1) * 128)
                    xt_st = xpool.tile([128, DK, 128], bf16, name="xt_st")
                    if st == 0:
                        # interleave x/weight chunks so the first matmuls
                        # unblock as soon as the leading chunks land
                        for c in range(16):
                            ksl = slice(c * 2, (c + 1) * 2)
                            nc.sync.dma_start(xt_st[:, ksl, :], xt[st][:, ksl, :])
                            nc.sync.dma_start(wq_sb[:, ksl, :], wq[:, ksl, :])
                            nc.sync.dma_start(wkv_sb[:, ksl, :], wkv[:, ksl, :])
                    else:
                        nc.sync.dma_start(xt_st[:], xt[st])

                    q_ps = p1psum.tile([128, QH * HD], f32, name="q_ps")
                    kv_ps = p1psum.tile([128, 2 * HD], f32, name="kv_ps")
                    for kt in range(DK):
                        nc.tensor.matmul(q_ps[:], lhsT=xt_st[:, kt], rhs=wq_sb[:, kt],
                                         start=(kt == 0), stop=(kt == DK - 1))
                        nc.tensor.matmul(kv_ps[:], lhsT=xt_st[:, kt], rhs=wkv_sb[:, kt],
                                         start=(kt == 0), stop=(kt == DK - 1))
                    if pending is not None:
                        emit_transposes(*pending)

                    # RoPE on q (4 heads batched) during PSUM eviction.
                    qp = q_ps[:].rearrange("p (h d) -> p h d", h=QH)
                    qa, qb = qp[:, :, 0:64], qp[:, :, 64:128]
                    cbc = cos_sb[:, st:st + 1, :].to_broadcast([128, QH, 64])
                    sbc = sin_sb[:, st:st + 1, :].to_broadcast([128, QH, 64])
                    t1 = p1tmp.tile([128, QH, 64], f32, name="t1")
                    t2 = p1tmp.tile([128, QH, 64], f32, name="t2")
                    qnat = p1tmp.tile([128, QH, HD], bf16, name="qnat")
                    na, nb = qnat[:, :, 0:64], qnat[:, :, 64:128]
                    nc.vector.tensor_tensor(t1[:], qa, sbc, mybir.AluOpType.mult)
                    nc.vector.tensor_tensor(t2[:], qb, sbc, mybir.AluOpType.mult)
                    nc.vector.tensor_tensor(na, qa, cbc, mybir.AluOpType.mult)
                    nc.vector.tensor_tensor(nb, qb, cbc, mybir.AluOpType.mult)
                    nc.vector.tensor_tensor(na, na, t2[:], mybir.AluOpType.subtract)
                    nc.vector.tensor_tensor(nb, nb, t1[:], mybir.AluOpType.add)

                    # RoPE on k
                    ka, kb = kv_ps[:, 0:64], kv_ps[:, 64:128]
                    cb1 = cos_sb[:, st, :]
                    sb1 = sin_sb[:, st, :]
                    kt1 = p1tmp.tile([128, 64], f32, name="kt1")
                    kt2 = p1tmp.tile([128, 64], f32, name="kt2")
                    knat = p1tmp.tile([128, HD], bf16, name="knat")
                    kna, knb = knat[:, 0:64], knat[:, 64:128]
                    nc.vector.tensor_tensor(kt1[:], ka, sb1, mybir.AluOpType.mult)
                    nc.vector.tensor_tensor(kt2[:], kb, sb1, mybir.AluOpType.mult)
                    nc.vector.tensor_tensor(kna, ka, cb1, mybir.AluOpType.mult)
                    nc.vector.tensor_tensor(knb, kb, cb1, mybir.AluOpType.mult)
                    nc.vector.tensor_tensor(kna, kna, kt2[:], mybir.AluOpType.subtract)
                    nc.vector.tensor_tensor(knb, knb, kt1[:], mybir.AluOpType.add)

                    # V natural, straight copy
                    nc.vector.tensor_copy(v_sb[:, st, :], kv_ps[:, HD:2 * HD])

                    pending = (qnat, knat, ssl)
                emit_transposes(*pending)

            # pools that live through phases 2 and 3
            with (
                tc.tile_pool(name="wopool", bufs=1) as wopool,
                tc.tile_pool(name="ohpool", bufs=2) as ohpool,
                tc.tile_pool(name="yaccp", bufs=1) as yaccp,
            ):
                wo_sb = wopool.tile([128, DK, 512], bf16)
                nc.gpsimd.dma_start(wo_sb[:], wo)
                y_acc = yaccp.tile([128, 4, 4, SG], f32)
                oh_tiles = []

                # ---------------- Phase 2: attention per (head, group)
                with (
                    tc.tile_pool(name="p2tmp", bufs=3) as p2tmp,
                    tc.tile_pool(name="p2lb", bufs=2) as p2lb,
                    tc.tile_pool(name="p2psum", bufs=2, space="PSUM") as p2psum,
                    tc.tile_pool(name="p2opsum", bufs=2, space="PSUM") as p2opsum,
                ):
                    def finalize(fin):
                        # normalization chain for a finished group, emitted
                        # after the NEXT group's score prologue so the lb
                        # matmul never heads the PE queue while the DVE
                        # reciprocal chain is still in flight
                        fh, fgsl, fot, fl, last = fin
                        linv_f = p2lb.tile([1, SG], f32, name="linv_f")
                        nc.vector.reciprocal_approx_fast(linv_f[:], fl[:])
                        linv_r = p2lb.tile([1, SG], f32r, name="linv_r")
                        nc.vector.tensor_copy(linv_r[:], linv_f[:])
                        lb_ps = p2opsum.tile([128, SG], f32, name="lb_ps",
                                             tag="lnorm", bufs=1)
                        nc.tensor.matmul(lb_ps[:], lhsT=onesr_sb[:],
                                         rhs=linv_r[:], start=True, stop=True)
                        lb_sb = p2lb.tile([128, SG], f32, name="lb_sb")
                        nc.vector.tensor_copy(lb_sb[:], lb_ps[:])
                        on_sb = p2tmp.tile([128, SG], bf16, name="on_sb")
                        nc.vector.tensor_tensor(on_sb[:], fot[:], lb_sb[:],
                                                mybir.AluOpType.mult)
                        nc.sync.dma_start(cc_in[fh][:, fgsl], on_sb[:])
                        if last:
                            nc.gpsimd.collective_compute(
                                "AllGather", mybir.AluOpType.bypass,
                                ins=[cc_in[fh].opt()], outs=[cc_out[fh].opt()],
                                replica_groups=[list(range(NCORES))],
                            )
                            # stage this head's gathered O into SBUF right
                            # away so the output projection can start the
                            # moment the PE queue drains phase 2. gpsimd
                            # queue: idle after startup, so the AG-wait here
                            # cannot stall the sync queue's qt_g prefetches
                            oh = ohpool.tile([128, NCORES, S], bf16, name="oh",
                                             tag="oh")
                            nc.gpsimd.dma_start(
                                oh[:],
                                cc_out[fh].rearrange("(r p) s -> p r s", p=128))
                            oh_tiles.append(oh)

                    pending_fin = None
                    for h in range(QH):
                        for g in range(G):
                            gsl = slice(g * SG, (g + 1) * SG)
                            qt_g = p2tmp.tile([128, SG], bf16, name="qt_g", bufs=4)
                            nc.sync.dma_start(
                                qt_g[:], qt_spill[h * 128:(h + 1) * 128, gsl])

                            nk = 4 * g + 4
                            DEPTH = 5
                            st_tiles = {}

                            def off_of(j):
                                # fully-masked q columns skipped on diagonal tiles
                                return max(0, (j - 4 * g)) * 128

                            def do_st(j, qt_g=qt_g, st_tiles=st_tiles,
                                      off_of=off_of):
                                off = off_of(j)
                                stp = p2psum.tile([128, SG], f32, name="st_ps",
                                                  tag="st_ps", bufs=DEPTH)
                                nc.tensor.matmul(
                                    stp[:, off:], lhsT=kt_sb[:, j * 128:(j + 1) * 128],
                                    rhs=qt_g[:, off:], start=True, stop=True)
                                if j >= 4 * g:
                                    # additive causal mask applied in PSUM,
                                    # ahead of the exp -> AV critical path
                                    nc.vector.tensor_tensor(
                                        stp[:, off:off + 128],
                                        stp[:, off:off + 128], madd_sb[:],
                                        mybir.AluOpType.add)
                                st_tiles[j] = stp

                            for j in range(min(DEPTH, nk)):
                                do_st(j)
                            if pending_fin is not None:
                                finalize(pending_fin)

                            ot_ps = p2opsum.tile([128, SG], f32, name="ot_ps", bufs=2)
                            l_ps = p2opsum.tile([1, SG], f32, name="l_ps",
                                                tag="lnorm", bufs=1)
                            for j in range(nk):
                                off = off_of(j)
                                st_ps = st_tiles.pop(j)
                                put = p2tmp.tile([128, SG], bf16, name="put", bufs=6)
                                nc.scalar.activation(put[:, off:], st_ps[:, off:],
                                                     mybir.ActivationFunctionType.Exp,
                                                     scale=SCALE)
                                nc.tensor.matmul(ot_ps[:, off:], lhsT=v_sb[:, j, :],
                                                 rhs=put[:, off:],
                                                 start=(j == 0), stop=(j == nk - 1),
                                                 skip_group_check=True)
                                nc.tensor.matmul(l_ps[:, off:], lhsT=onesc_sb[:],
                                                 rhs=put[:, off:],
                                                 start=(j == 0), stop=(j == nk - 1),
                                                 skip_group_check=True)
                                if j + DEPTH < nk:
                                    do_st(j + DEPTH)

                            pending_fin = (h, gsl, ot_ps, l_ps, g == G - 1)
                    finalize(pending_fin)

                # ---------------- Phase 3: yT += wo_h^T @ O_h^T per head
                with (
                    tc.tile_pool(name="p3out", bufs=3) as p3out,
                    tc.tile_pool(name="p3psum", bufs=2, space="PSUM") as p3psum,
                ):
                    for h in range(QH):
                        oh = oh_tiles[h]
                        for dt in range(4):
                            y_ps = p3psum.tile([128, 4, SG], f32, name="y_ps",
                                               bufs=2)
                            for r in range(NCORES):
                                kt2 = h * NCORES + r
                                for sq in range(4):
                                    nc.tensor.matmul(
                                        y_ps[:, sq, :],
                                        lhsT=wo_sb[:, kt2,
                                                   dt * 128:(dt + 1) * 128],
                                        rhs=oh[:, r, sq * SG:(sq + 1) * SG],
                                        start=(r == 0), stop=(r == NCORES - 1))
                            for sq in range(4):
                                if h == 0:
                                    nc.vector.tensor_copy(
                                        y_acc[:, dt, sq, :], y_ps[:, sq, :])
                                elif h < QH - 1:
                                    nc.vector.tensor_tensor(
                                        y_acc[:, dt, sq, :],
                                        y_acc[:, dt, sq, :], y_ps[:, sq, :],
                                        mybir.AluOpType.add)
                                else:
                                    y_fin = p3out.tile([128, SG], f32,
                                                       name="y_fin")
                                    nc.vector.tensor_tensor(
                                        y_fin[:], y_acc[:, dt, sq, :],
                                        y_ps[:, sq, :], mybir.AluOpType.add)
                                    nc.sync.dma_start(
                                        yt[dt * 128:(dt + 1) * 128,
                                           sq * SG:(sq + 1) * SG], y_fin[:])
    nc.compile()
    return nc


def _host_prep(inputs):
    bf = ml_dtypes.bfloat16
    x = np.asarray(inputs["x"], dtype=np.float32)
    wq = np.asarray(inputs["wq"], dtype=np.float32)
    wk = np.asarray(inputs["wk"], dtype=np.float32)
    wv = np.asarray(inputs["wv"], dtype=np.float32)
    wo = np.asarray(inputs["wo"], dtype=np.float32)
    cos = np.asarray(inputs["freqs_cos"], dtype=np.float32)
    sin = np.asarray(inputs["freqs_sin"], dtype=np.float32)
    mask = np.asarray(inputs["mask"], dtype=np.float32)

    # xt[st, p, kt, s] = x[128*st + s, 128*kt + p]
    xts = np.ascontiguousarray(
        x.reshape(ST, 128, DK, 128).transpose(0, 3, 2, 1)).astype(bf)

    # de-interleave RoPE pairs within each head: evens then odds
    perm = np.concatenate([np.arange(0, HD, 2), np.arange(1, HD, 2)])

    cos_t = np.ascontiguousarray(cos.reshape(ST, 128, 64).transpose(1, 0, 2))
    sin_t = np.ascontiguousarray(sin.reshape(ST, 128, 64).transpose(1, 0, 2))

    # additive causal mask for a diagonal 128x128 tile [k-in-tile, q-in-tile],
    # derived from the actual mask input (g-independent for causal)
    madd_t = np.ascontiguousarray(
        np.where(mask[0:128, 0:128].T == 0.0, 0.0, -30000.0)).astype(np.float32)

    def ktile(w):  # [D, m] -> [128, DK, m]
        return np.ascontiguousarray(
            w.reshape(DK, 128, w.shape[1]).transpose(1, 0, 2))

    in_maps = []
    for c in range(NCORES):
        wq_c = wq[:, 512 * c:512 * (c + 1)].reshape(D, QH, HD)[:, :, perm]
        wq_c = wq_c.reshape(D, QH * HD)
        wk_c = wk[:, 128 * c:128 * (c + 1)][:, perm]
        wv_c = wv[:, 128 * c:128 * (c + 1)]
        wkv_c = np.concatenate([wk_c, wv_c], axis=1)
        # wo rows reordered to (head, rank, d) to match per-head AllGather
        wo_c = wo[:, 512 * c:512 * (c + 1)]
        wo_c = wo_c.reshape(NCORES, QH, 128, 512).transpose(1, 0, 2, 3)
        wo_c = wo_c.reshape(D, 512)
        in_maps.append({
            "xt": xts,
            "wq": ktile(wq_c).astype(bf),
            "wkv": ktile(wkv_c).astype(bf),
            "wo": ktile(wo_c).astype(bf),
            "cos": cos_t,
            "sin": sin_t,
            "madd": madd_t,
            "onesc": np.ones((128, 1), dtype=bf),
            "onesr": np.ones((1, 128), dtype=np.float32),
            "ident": np.eye(128, dtype=bf),
        })
    return in_maps


def _run(inputs, trace=False, tmpdir=None):
    from concourse import bass_utils
    if "nc" not in _CACHE:
        _CACHE["nc"] = _build()
    nc = _CACHE["nc"]
    in_maps = _host_prep(inputs)
    res = bass_utils.run_bass_kernel_spmd(
        nc, in_maps, core_ids=list(range(NCORES)), trace=trace, tmpdir=tmpdir)
    yts = [res.results[c]["yt"] for c in range(NCORES)]
    y = np.concatenate([t.T for t in yts], axis=1).astype(np.float32)
    return y.reshape(1, S, D), res


def kernel(**inputs):
    y, _ = _run(inputs, trace=False)
    return y


# revision 29
# speedup vs baseline: 1.0378x; 1.0219x over previous
"""GQA attention layer (B=1, S=2048, D=4096, H=32, KV=8, HD=128) on 8 TRN2
NeuronCores. Tensor-parallel over the 8 KV-head groups; per-head AllGather of
the attention outputs (overlapped with attention compute), then each core
computes one column shard of the output projection (no AllReduce needed).

Datapath is bf16 into fp32 PSUM accumulation (x/wq/wkv/q/k/v/P/O/wo in bf16;
scores, softmax sums and the output projection accumulate in fp32). Attention
is computed in transposed orientation (scores^T = kT-slices as lhsT against
qT) so softmax sums become TensorEngine matmuls and no P-transposes are
needed. Exp runs on paired score tiles (1024-wide) to amortize ACT overhead.
The output projection accumulates per-head partials into an SBUF fp32
accumulator so each head's matmuls start as soon as its AllGather lands.
RoPE pairs are de-interleaved by permuting wq/wk columns host-side; all DRAM
operands are host-pretiled for contiguous DMA.
"""
import numpy as np
import ml_dtypes

S = 2048
D = 4096
HD = 128
QH = 4            # q heads per core
NCORES = 8
ST = S // 128     # 16 s-tiles
DK = D // 128     # 32 contraction tiles
G = 4             # q groups per head
SG = S // G       # 512 q positions per group
SCALE = 1.0 / np.sqrt(128.0)

_CACHE = {}


def _build():
    import concourse.mybir as mybir
    import concourse.tile as tile
    from concourse import bacc

    f32, f32r = mybir.dt.float32, mybir.dt.float32r
    bf16 = mybir.dt.bfloat16
    nc = bacc.Bacc("TRN2", target_bir_lowering=False, debug=False,
                   num_devices=NCORES)

    xt = nc.dram_tensor("xt", [ST, 128, DK, 128], bf16, kind="ExternalInput").ap()
    wq = nc.dram_tensor("wq", [128, DK, QH * HD], bf16, kind="ExternalInput").ap()
    wkv = nc.dram_tensor("wkv", [128, DK, 2 * HD], bf16, kind="ExternalInput").ap()
    wo = nc.dram_tensor("wo", [128, DK, 512], bf16, kind="ExternalInput").ap()
    cos = nc.dram_tensor("cos", [128, ST, 64], f32, kind="ExternalInput").ap()
    sin = nc.dram_tensor("sin", [128, ST, 64], f32, kind="ExternalInput").ap()
    madd = nc.dram_tensor("madd", [128, 128], f32, kind="ExternalInput").ap()
    onesc = nc.dram_tensor("onesc", [128, 1], bf16, kind="ExternalInput").ap()
    onesr = nc.dram_tensor("onesr", [1, 128], f32r, kind="ExternalInput").ap()
    ident = nc.dram_tensor("ident", [128, 128], bf16, kind="ExternalInput").ap()
    yt = nc.dram_tensor("yt", [512, S], f32, kind="ExternalOutput").ap()

    with tile.TileContext(nc) as tc:
        with (
            tc.tile_pool(name="const", bufs=1) as constp,
            tc.tile_pool(name="resid", bufs=1) as resid,
            tc.tile_pool(name="dram", bufs=1, space="DRAM") as dram,
        ):
            cos_sb = constp.tile([128, ST, 64], f32)
            sin_sb = constp.tile([128, ST, 64], f32)
            madd_sb = constp.tile([128, 128], f32)
            onesc_sb = constp.tile([128, 1], bf16)
            onesr_sb = constp.tile([1, 128], f32r)
            ident_sb = constp.tile([128, 128], bf16)
            nc.gpsimd.dma_start(cos_sb[:], cos)
            nc.gpsimd.dma_start(sin_sb[:], sin)
            nc.gpsimd.dma_start(madd_sb[:], madd)
            nc.gpsimd.dma_start(onesc_sb[:], onesc)
            nc.gpsimd.dma_start(onesr_sb[:], onesr)
            nc.gpsimd.dma_start(ident_sb[:], ident)

            # residents across phases
            kt_sb = resid.tile([128, S], bf16)           # roped K^T  [d, s]
            v_sb = resid.tile([128, ST, HD], bf16)       # natural V  [s-in-tile, st, d]

            qt_spill = dram.tile([QH * 128, S], bf16)    # roped Q^T per head
            cc_in = [dram.tile([128, S], bf16, name=f"cc_in{h}") for h in range(QH)]
            cc_out = [dram.tile([NCORES * 128, S], bf16, addr_space="Shared",
                                name=f"cc_out{h}") for h in range(QH)]

            # ---------------- Phase 1: QKV projections + RoPE + transposes
            with (
                tc.tile_pool(name="wpool", bufs=1) as wpool,
                tc.tile_pool(name="xpool", bufs=3) as xpool,
                tc.tile_pool(name="p1tmp", bufs=3) as p1tmp,
                tc.tile_pool(name="p1psum", bufs=2, space="PSUM") as p1psum,
            ):
                wq_sb = wpool.tile([128, DK, QH * HD], bf16)
                wkv_sb = wpool.tile([128, DK, 2 * HD], bf16)

                def emit_transposes(qnat, knat, ssl):
                    # deferred one s-tile so PE never waits on the DVE rope
                    qts = p1tmp.tile([128, QH, 128], bf16, name="qts")
                    for h in range(QH):
                        qtr = p1psum.tile([128, 128], bf16, name="qtr")
                        nc.tensor.transpose(qtr[:], qnat[:, h, :], ident_sb[:])
                        nc.vector.tensor_copy(qts[:, h, :], qtr[:])
                    nc.sync.dma_start(
                        qt_spill.rearrange("(h p) s -> p h s", p=128)[:, :, ssl],
                        qts[:])
                    ktr = p1psum.tile([128, 128], bf16, name="ktr")
                    nc.tensor.transpose(ktr[:], knat[:], ident_sb[:])
                    nc.vector.tensor_copy(kt_sb[:, ssl], ktr[:])

                pending = None
                for st in range(ST):
                    ssl = slice(st * 128, (st + 1) * 128)
                    xt_st = xpool.tile([128, DK, 128], bf16, name="xt_st")
                    if st == 0:
                        # interleave x/weight chunks so the first matmuls
                        # unblock as soon as the leading chunks land
                        for c in range(8):
                            ksl = slice(c * 4, (c + 1) * 4)
                            nc.sync.dma_start(xt_st[:, ksl, :], xt[st][:, ksl, :])
                            nc.sync.dma_start(wq_sb[:, ksl, :], wq[:, ksl, :])
                            nc.sync.dma_start(wkv_sb[:, ksl, :], wkv[:, ksl, :])
                    else:
                        nc.sync.dma_start(xt_st[:], xt[st])

                    q_ps = p1psum.tile([128, QH * HD], f32, name="q_ps")
                    kv_ps = p1psum.tile([128, 2 * HD], f32, name="kv_ps")
                    for kt in range(DK):
                        nc.tensor.matmul(q_ps[:], lhsT=xt_st[:, kt], rhs=wq_sb[:, kt],
                                         start=(kt == 0), stop=(kt == DK - 1))
                        nc.tensor.matmul(kv_ps[:], lhsT=xt_st[:, kt], rhs=wkv_sb[:, kt],
                                         start=(kt == 0), stop=(kt == DK - 1))
                    if pending is not None:
                        emit_transposes(*pending)

                    # RoPE on q (4 heads batched) during PSUM eviction.
                    qp = q_ps[:].rearrange("p (h d) -> p h d", h=QH)
                    qa, qb = qp[:, :, 0:64], qp[:, :, 64:128]
                    cbc = cos_sb[:, st:st + 1, :].to_broadcast([128, QH, 64])
                    sbc = sin_sb[:, st:st + 1, :].to_broadcast([128, QH, 64])
                    t1 = p1tmp.tile([128, QH, 64], f32, name="t1")
                    t2 = p1tmp.tile([128, QH, 64], f32, name="t2")
                    qnat = p1tmp.tile([128, QH, HD], bf16, name="qnat")
                    na, nb = qnat[:, :, 0:64], qnat[:, :, 64:128]
                    nc.vector.tensor_tensor(t1[:], qa, sbc, mybir.AluOpType.mult)
                    nc.vector.tensor_tensor(t2[:], qb, sbc, mybir.AluOpType.mult)
                    nc.vector.tensor_tensor(na, qa, cbc, mybir.AluOpType.mult)
                    nc.vector.tensor_tensor(nb, qb, cbc, mybir.AluOpType.mult)
                    nc.vector.tensor_tensor(na, na, t2[:], mybir.AluOpType.subtract)
                    nc.vector.tensor_tensor(nb, nb, t1[:], mybir.AluOpType.add)

                    # RoPE on k
                    ka, kb = kv_ps[:, 0:64], kv_ps[:, 64:128]
                    cb1 = cos_sb[:, st, :]
                    sb1 = sin_sb[:, st, :]
                    kt1 = p1tmp.tile([128, 64], f32, name="kt1")
                    kt2 = p1tmp.tile([128, 64], f32, name="kt2")
                    knat = p1tmp.tile([128, HD], bf16, name="knat")
                    kna, knb = knat[:, 0:64], knat[:, 64:128]
                    nc.vector.tensor_tensor(kt1[:], ka, sb1, mybir.AluOpType.mult)
                    nc.vector.tensor_tensor(kt2[:], kb, sb1, mybir.AluOpType.mult)
                    nc.vector.tensor_tensor(kna, ka, cb1, mybir.AluOpType.mult)
                    nc.vector.tensor_tensor(knb, kb, cb1, mybir.AluOpType.mult)
                    nc.vector.tensor_tensor(kna, kna, kt2[:], mybir.AluOpType.subtract)
                    nc.vector.tensor_tensor(knb, knb, kt1[:], mybir.AluOpType.add)

                    # V natural, straight copy
                    nc.vector.tensor_copy(v_sb[:, st, :], kv_ps[:, HD:2 * HD])

                    pending = (qnat, knat, ssl)
                emit_transposes(*pending)

            # pools that live through phases 2 and 3
            with (
                tc.tile_pool(name="wopool", bufs=1) as wopool,
                tc.tile_pool(name="ohpool", bufs=2) as ohpool,
                tc.tile_pool(name="yaccp", bufs=1) as yaccp,
            ):
                wo_sb = wopool.tile([128, DK, 512], bf16)
                nc.gpsimd.dma_start(wo_sb[:], wo)
                y_acc = yaccp.tile([128, 4, 4, SG], f32)
                oh_tiles = []

                # ---------------- Phase 2: attention per (head, group)
                with (
                    tc.tile_pool(name="p2tmp", bufs=3) as p2tmp,
                    tc.tile_pool(name="p2lb", bufs=2) as p2lb,
                    tc.tile_pool(name="p2psum", bufs=2, space="PSUM") as p2psum,
                    tc.tile_pool(name="p2opsum", bufs=2, space="PSUM") as p2opsum,
                ):
                    def finalize(fin):
                        # normalization chain for a finished group, emitted
                        # after the NEXT group's score prologue so the lb
                        # matmul never heads the PE queue while the DVE
                        # reciprocal chain is still in flight
                        fh, fgsl, fot, fl, last = fin
                        linv_f = p2lb.tile([1, SG], f32, name="linv_f")
                        nc.vector.reciprocal_approx_fast(linv_f[:], fl[:])
                        linv_r = p2lb.tile([1, SG], f32r, name="linv_r")
                        nc.vector.tensor_copy(linv_r[:], linv_f[:])
                        lb_ps = p2opsum.tile([128, SG], f32, name="lb_ps",
                                             tag="lnorm", bufs=1)
                        nc.tensor.matmul(lb_ps[:], lhsT=onesr_sb[:],
                                         rhs=linv_r[:], start=True, stop=True)
                        lb_sb = p2lb.tile([128, SG], f32, name="lb_sb")
                        nc.vector.tensor_copy(lb_sb[:], lb_ps[:])
                        on_sb = p2tmp.tile([128, SG], bf16, name="on_sb")
                        nc.vector.tensor_tensor(on_sb[:], fot[:], lb_sb[:],
                                                mybir.AluOpType.mult)
                        nc.sync.dma_start(cc_in[fh][:, fgsl], on_sb[:])
                        if last:
                            nc.gpsimd.collective_compute(
                                "AllGather", mybir.AluOpType.bypass,
                                ins=[cc_in[fh].opt()], outs=[cc_out[fh].opt()],
                                replica_groups=[list(range(NCORES))],
                            )
                            # stage this head's gathered O into SBUF right
                            # away so the output projection can start the
                            # moment the PE queue drains phase 2. gpsimd
                            # queue: idle after startup, so the AG-wait here
                            # cannot stall the sync queue's qt_g prefetches
                            oh = ohpool.tile([128, NCORES, S], bf16, name="oh",
                                             tag="oh")
                            nc.gpsimd.dma_start(
                                oh[:],
                                cc_out[fh].rearrange("(r p) s -> p r s", p=128))
                            oh_tiles.append(oh)

                    pending_fin = None
                    for h in range(QH):
                        for g in range(G):
                            gsl = slice(g * SG, (g + 1) * SG)
                            qt_g = p2tmp.tile([128, SG], bf16, name="qt_g", bufs=4)
                            nc.sync.dma_start(
                                qt_g[:], qt_spill[h * 128:(h + 1) * 128, gsl])

                            nk = 4 * g + 4
                            DEPTH = 5
                            st_tiles = {}

                            def off_of(j):
                                # fully-masked q columns skipped on diagonal tiles
                                return max(0, (j - 4 * g)) * 128

                            def do_st(j, qt_g=qt_g, st_tiles=st_tiles,
                                      off_of=off_of):
                                off = off_of(j)
                                stp = p2psum.tile([128, SG], f32, name="st_ps",
                                                  tag="st_ps", bufs=DEPTH)
                                nc.tensor.matmul(
                                    stp[:, off:], lhsT=kt_sb[:, j * 128:(j + 1) * 128],
                                    rhs=qt_g[:, off:], start=True, stop=True)
                                if j >= 4 * g:
                                    # additive causal mask applied in PSUM,
                                    # ahead of the exp -> AV critical path
                                    nc.vector.tensor_tensor(
                                        stp[:, off:off + 128],
                                        stp[:, off:off + 128], madd_sb[:],
                                        mybir.AluOpType.add)
                                st_tiles[j] = stp

                            for j in range(min(DEPTH, nk)):
                                do_st(j)
                            if pending_fin is not None:
                                finalize(pending_fin)

                            ot_ps = p2opsum.tile([128, SG], f32, name="ot_ps", bufs=2)
                            l_ps = p2opsum.tile([1, SG], f32, name="l_ps",
                                                tag="lnorm", bufs=1)
                            for j in range(nk):
                                off = off_of(j)
                                st_ps = st_tiles.pop(j)
                                put = p2tmp.tile([128, SG], bf16, name="put", bufs=6)
                                nc.scalar.activation(put[:, off:], st_ps[:, off:],
                                                     mybir.ActivationFunctionType.Exp,
                                                     scale=SCALE)
                                nc.tensor.matmul(ot_ps[:, off:], lhsT=v_sb[:, j, :],
                                                 rhs=put[:, off:],
                                                 start=(j == 0), stop=(j == nk - 1),
                                                 skip_group_check=True)
                                nc.tensor.matmul(l_ps[:, off:], lhsT=onesc_sb[:],
                                                 rhs=put[:, off:],
                                                 start=(j == 0), stop=(j == nk - 1),
                                                 skip_group_check=True)
                                if j + DEPTH < nk:
                                    do_st(j + DEPTH)

                            pending_fin = (h, gsl, ot_ps, l_ps, g == G - 1)
                    finalize(pending_fin)

                # ---------------- Phase 3: yT += wo_h^T @ O_h^T per head
                with (
                    tc.tile_pool(name="p3out", bufs=3) as p3out,
                    tc.tile_pool(name="p3psum", bufs=2, space="PSUM") as p3psum,
                ):
                    for h in range(QH):
                        oh = oh_tiles[h]
                        for dt in range(4):
                            y_ps = p3psum.tile([128, 4, SG], f32, name="y_ps",
                                               bufs=2)
                            for r in range(NCORES):
                                kt2 = h * NCORES + r
                                for sq in range(4):
                                    nc.tensor.matmul(
                                        y_ps[:, sq, :],
                                        lhsT=wo_sb[:, kt2,
                                                   dt * 128:(dt + 1) * 128],
                                        rhs=oh[:, r, sq * SG:(sq + 1) * SG],
                                        start=(r == 0), stop=(r == NCORES - 1))
                            for sq in range(4):
                                if h == 0:
                                    nc.vector.tensor_copy(
                                        y_acc[:, dt, sq, :], y_ps[:, sq, :])
                                elif h < QH - 1:
                                    nc.vector.tensor_tensor(
                                        y_acc[:, dt, sq, :],
                                        y_acc[:, dt, sq, :], y_ps[:, sq, :],
                                        mybir.AluOpType.add)
                                else:
                                    y_fin = p3out.tile([128, SG], f32,
                                                       name="y_fin")
                                    nc.vector.tensor_tensor(
                                        y_fin[:], y_acc[:, dt, sq, :],
                                        y_ps[:, sq, :], mybir.AluOpType.add)
                                    nc.sync.dma_start(
                                        yt[dt * 128:(dt + 1) * 128,
                                           sq * SG:(sq + 1) * SG], y_fin[:])
    nc.compile()
    return nc


def _host_prep(inputs):
    bf = ml_dtypes.bfloat16
    x = np.asarray(inputs["x"], dtype=np.float32)
    wq = np.asarray(inputs["wq"], dtype=np.float32)
    wk = np.asarray(inputs["wk"], dtype=np.float32)
    wv = np.asarray(inputs["wv"], dtype=np.float32)
    wo = np.asarray(inputs["wo"], dtype=np.float32)
    cos = np.asarray(inputs["freqs_cos"], dtype=np.float32)
    sin = np.asarray(inputs["freqs_sin"], dtype=np.float32)
    mask = np.asarray(inputs["mask"], dtype=np.float32)

    # xt[st, p, kt, s] = x[128*st + s, 128*kt + p]
    xts = np.ascontiguousarray(
        x.reshape(ST, 128, DK, 128).transpose(0, 3, 2, 1)).astype(bf)

    # de-interleave RoPE pairs within each head: evens then odds
    perm = np.concatenate([np.arange(0, HD, 2), np.arange(1, HD, 2)])

    cos_t = np.ascontiguousarray(cos.reshape(ST, 128, 64).transpose(1, 0, 2))
    sin_t = np.ascontiguousarray(sin.reshape(ST, 128, 64).transpose(1, 0, 2))

    # additive causal mask for a diagonal 128x128 tile [k-in-tile, q-in-tile],
    # derived from the actual mask input (g-independent for causal)
    madd_t = np.ascontiguousarray(
        np.where(mask[0:128, 0:128].T == 0.0, 0.0, -30000.0)).astype(np.float32)

    def ktile(w):  # [D, m] -> [128, DK, m]
        return np.ascontiguousarray(
            w.reshape(DK, 128, w.shape[1]).transpose(1, 0, 2))

    in_maps = []
    for c in range(NCORES):
        wq_c = wq[:, 512 * c:512 * (c + 1)].reshape(D, QH, HD)[:, :, perm]
        wq_c = wq_c.reshape(D, QH * HD)
        wk_c = wk[:, 128 * c:128 * (c + 1)][:, perm]
        wv_c = wv[:, 128 * c:128 * (c + 1)]
        wkv_c = np.concatenate([wk_c, wv_c], axis=1)
        # wo rows reordered to (head, rank, d) to match per-head AllGather
        wo_c = wo[:, 512 * c:512 * (c + 1)]
        wo_c = wo_c.reshape(NCORES, QH, 128, 512).transpose(1, 0, 2, 3)
        wo_c = wo_c.reshape(D, 512)
        in_maps.append({
            "xt": xts,
            "wq": ktile(wq_c).astype(bf),
            "wkv": ktile(wkv_c).astype(bf),
            "wo": ktile(wo_c).astype(bf),
            "cos": cos_t,
            "sin": sin_t,
            "madd": madd_t,
            "onesc": np.ones((128, 1), dtype=bf),
            "onesr": np.ones((1, 128), dtype=np.float32),
            "ident": np.eye(128, dtype=bf),
        })
    return in_maps


def _run(inputs, trace=False, tmpdir=None):
    from concourse import bass_utils
    if "nc" not in _CACHE:
        _CACHE["nc"] = _build()
    nc = _CACHE["nc"]
    in_maps = _host_prep(inputs)
    res = bass_utils.run_bass_kernel_spmd(
        nc, in_maps, core_ids=list(range(NCORES)), trace=trace, tmpdir=tmpdir)
    yts = [res.results[c]["yt"] for c in range(NCORES)]
    y = np.concatenate([t.T for t in yts], axis=1).astype(np.float32)
    return y.reshape(1, S, D), res


def kernel(**inputs):
    y, _ = _run(inputs, trace=False)
    return y


# revision 32
# speedup vs baseline: 1.0558x; 1.0174x over previous
"""GQA attention layer (B=1, S=2048, D=4096, H=32, KV=8, HD=128) on 8 TRN2
NeuronCores. Tensor-parallel over the 8 KV-head groups; per-head AllGather of
the attention outputs (overlapped with attention compute), then each core
computes one column shard of the output projection (no AllReduce needed).

Datapath is bf16 into fp32 PSUM accumulation (x/wq/wkv/q/k/v/P/O/wo in bf16;
scores, softmax sums and the output projection accumulate in fp32). Attention
is computed in transposed orientation (scores^T = kT-slices as lhsT against
qT) so softmax sums become TensorEngine matmuls and no P-transposes are
needed. Exp runs on paired score tiles (1024-wide) to amortize ACT overhead.
The output projection accumulates per-head partials into an SBUF fp32
accumulator so each head's matmuls start as soon as its AllGather lands.
RoPE pairs are de-interleaved by permuting wq/wk columns host-side; all DRAM
operands are host-pretiled for contiguous DMA.
"""
import numpy as np
import ml_dtypes

S = 2048
D = 4096
HD = 128
QH = 4            # q heads per core
NCORES = 8
ST = S // 128     # 16 s-tiles
DK = D // 128     # 32 contraction tiles
G = 4             # q groups per head
SG = S // G       # 512 q positions per group
SCALE = 1.0 / np.sqrt(128.0)

_CACHE = {}


def _build():
    import concourse.mybir as mybir
    import concourse.tile as tile
    from concourse import bacc

    f32, f32r = mybir.dt.float32, mybir.dt.float32r
    bf16 = mybir.dt.bfloat16
    nc = bacc.Bacc("TRN2", target_bir_lowering=False, debug=False,
                   num_devices=NCORES)

    xt = nc.dram_tensor("xt", [ST, 128, DK, 128], bf16, kind="ExternalInput").ap()
    wq = nc.dram_tensor("wq", [128, DK, QH * HD], bf16, kind="ExternalInput").ap()
    wkv = nc.dram_tensor("wkv", [128, DK, 2 * HD], bf16, kind="ExternalInput").ap()
    wo = nc.dram_tensor("wo", [128, DK, 512], bf16, kind="ExternalInput").ap()
    cos = nc.dram_tensor("cos", [128, ST, 64], f32, kind="ExternalInput").ap()
    sin = nc.dram_tensor("sin", [128, ST, 64], f32, kind="ExternalInput").ap()
    madd = nc.dram_tensor("madd", [128, 128], f32, kind="ExternalInput").ap()
    onesc = nc.dram_tensor("onesc", [128, 1], bf16, kind="ExternalInput").ap()
    onesr = nc.dram_tensor("onesr", [1, 128], f32r, kind="ExternalInput").ap()
    ident = nc.dram_tensor("ident", [128, 128], bf16, kind="ExternalInput").ap()
    yt = nc.dram_tensor("yt", [512, S], f32, kind="ExternalOutput").ap()

    with tile.TileContext(nc) as tc:
        with (
            tc.tile_pool(name="const", bufs=1) as constp,
            tc.tile_pool(name="resid", bufs=1) as resid,
            tc.tile_pool(name="dram", bufs=1, space="DRAM") as dram,
        ):
            cos_sb = constp.tile([128, ST, 64], f32)
            sin_sb = constp.tile([128, ST, 64], f32)
            madd_sb = constp.tile([128, 128], f32)
            onesc_sb = constp.tile([128, 1], bf16)
            onesr_sb = constp.tile([1, 128], f32r)
            ident_sb = constp.tile([128, 128], bf16)
            nc.gpsimd.dma_start(cos_sb[:], cos)
            nc.gpsimd.dma_start(sin_sb[:], sin)
            nc.gpsimd.dma_start(madd_sb[:], madd)
            nc.gpsimd.dma_start(onesc_sb[:], onesc)
            nc.gpsimd.dma_start(onesr_sb[:], onesr)
            nc.gpsimd.dma_start(ident_sb[:], ident)

            # residents across phases
            kt_sb = resid.tile([128, S], bf16)           # roped K^T  [d, s]
            v_sb = resid.tile([128, ST, HD], bf16)       # natural V  [s-in-tile, st, d]

            qt_spill = dram.tile([QH * 128, S], bf16)    # roped Q^T per head
            cc_in = [dram.tile([128, S], bf16, name=f"cc_in{h}") for h in range(QH)]
            cc_out = [dram.tile([NCORES * 128, S], bf16, addr_space="Shared",
                                name=f"cc_out{h}") for h in range(QH)]

            # ---------------- Phase 1: QKV projections + RoPE + transposes
            with (
                tc.tile_pool(name="wpool", bufs=1) as wpool,
                tc.tile_pool(name="xpool", bufs=3) as xpool,
                tc.tile_pool(name="p1tmp", bufs=3) as p1tmp,
                tc.tile_pool(name="p1psum", bufs=2, space="PSUM") as p1psum,
            ):
                wq_sb = wpool.tile([128, DK, QH * HD], bf16)
                wkv_sb = wpool.tile([128, DK, 2 * HD], bf16)

                def emit_transposes(qnat, knat, ssl):
                    # deferred one s-tile so PE never waits on the DVE rope
                    qts = p1tmp.tile([128, QH, 128], bf16, name="qts")
                    for h in range(QH):
                        qtr = p1psum.tile([128, 128], bf16, name="qtr")
                        nc.tensor.transpose(qtr[:], qnat[:, h, :], ident_sb[:])
                        nc.vector.tensor_copy(qts[:, h, :], qtr[:])
                    nc.sync.dma_start(
                        qt_spill.rearrange("(h p) s -> p h s", p=128)[:, :, ssl],
                        qts[:])
                    ktr = p1psum.tile([128, 128], bf16, name="ktr")
                    nc.tensor.transpose(ktr[:], knat[:], ident_sb[:])
                    nc.vector.tensor_copy(kt_sb[:, ssl], ktr[:])

                pending = None
                for st in range(ST):
                    ssl = slice(st * 128, (st + 1) * 128)
                    xt_st = xpool.tile([128, DK, 128], bf16, name="xt_st")
                    if st == 0:
                        # interleave x/weight chunks so the first matmuls
                        # unblock as soon as the leading chunks land
                        for c in range(8):
                            ksl = slice(c * 4, (c + 1) * 4)
                            nc.sync.dma_start(xt_st[:, ksl, :], xt[st][:, ksl, :])
                            nc.sync.dma_start(wq_sb[:, ksl, :], wq[:, ksl, :])
                            nc.sync.dma_start(wkv_sb[:, ksl, :], wkv[:, ksl, :])
                    else:
                        nc.sync.dma_start(xt_st[:], xt[st])

                    q_ps = p1psum.tile([128, QH * HD], f32, name="q_ps")
                    kv_ps = p1psum.tile([128, 2 * HD], f32, name="kv_ps")
                    for kt in range(DK):
                        nc.tensor.matmul(q_ps[:], lhsT=xt_st[:, kt], rhs=wq_sb[:, kt],
                                         start=(kt == 0), stop=(kt == DK - 1))
                        nc.tensor.matmul(kv_ps[:], lhsT=xt_st[:, kt], rhs=wkv_sb[:, kt],
                                         start=(kt == 0), stop=(kt == DK - 1))
                    if pending is not None:
                        emit_transposes(*pending)

                    # RoPE on q (4 heads batched) during PSUM eviction.
                    qp = q_ps[:].rearrange("p (h d) -> p h d", h=QH)
                    qa, qb = qp[:, :, 0:64], qp[:, :, 64:128]
                    cbc = cos_sb[:, st:st + 1, :].to_broadcast([128, QH, 64])
                    sbc = sin_sb[:, st:st + 1, :].to_broadcast([128, QH, 64])
                    t1 = p1tmp.tile([128, QH, 64], f32, name="t1")
                    t2 = p1tmp.tile([128, QH, 64], f32, name="t2")
                    qnat = p1tmp.tile([128, QH, HD], bf16, name="qnat")
                    na, nb = qnat[:, :, 0:64], qnat[:, :, 64:128]
                    nc.vector.tensor_tensor(t1[:], qa, sbc, mybir.AluOpType.mult)
                    nc.vector.tensor_tensor(t2[:], qb, sbc, mybir.AluOpType.mult)
                    nc.vector.tensor_tensor(na, qa, cbc, mybir.AluOpType.mult)
                    nc.vector.tensor_tensor(nb, qb, cbc, mybir.AluOpType.mult)
                    nc.vector.tensor_tensor(na, na, t2[:], mybir.AluOpType.subtract)
                    nc.vector.tensor_tensor(nb, nb, t1[:], mybir.AluOpType.add)

                    # RoPE on k
                    ka, kb = kv_ps[:, 0:64], kv_ps[:, 64:128]
                    cb1 = cos_sb[:, st, :]
                    sb1 = sin_sb[:, st, :]
                    kt1 = p1tmp.tile([128, 64], f32, name="kt1")
                    kt2 = p1tmp.tile([128, 64], f32, name="kt2")
                    knat = p1tmp.tile([128, HD], bf16, name="knat")
                    kna, knb = knat[:, 0:64], knat[:, 64:128]
                    nc.vector.tensor_tensor(kt1[:], ka, sb1, mybir.AluOpType.mult)
                    nc.vector.tensor_tensor(kt2[:], kb, sb1, mybir.AluOpType.mult)
                    nc.vector.tensor_tensor(kna, ka, cb1, mybir.AluOpType.mult)
                    nc.vector.tensor_tensor(knb, kb, cb1, mybir.AluOpType.mult)
                    nc.vector.tensor_tensor(kna, kna, kt2[:], mybir.AluOpType.subtract)
                    nc.vector.tensor_tensor(knb, knb, kt1[:], mybir.AluOpType.add)

                    # V natural, straight copy
                    nc.vector.tensor_copy(v_sb[:, st, :], kv_ps[:, HD:2 * HD])

                    pending = (qnat, knat, ssl)
                emit_transposes(*pending)

            # pools that live through phases 2 and 3
            with (
                tc.tile_pool(name="wopool", bufs=1) as wopool,
                tc.tile_pool(name="ohpool", bufs=2) as ohpool,
                tc.tile_pool(name="yaccp", bufs=1) as yaccp,
            ):
                wo_sb = wopool.tile([128, DK, 512], bf16)
                nc.gpsimd.dma_start(wo_sb[:], wo)
                y_acc = yaccp.tile([128, 4, 4, SG], f32)
                oh_tiles = []

                # ---------------- Phase 2: attention per (head, group)
                with (
                    tc.tile_pool(name="p2tmp", bufs=3) as p2tmp,
                    tc.tile_pool(name="p2lb", bufs=2) as p2lb,
                    tc.tile_pool(name="p2psum", bufs=2, space="PSUM") as p2psum,
                    tc.tile_pool(name="p2opsum", bufs=2, space="PSUM") as p2opsum,
                ):
                    def finalize(fin):
                        # normalization chain for a finished group, emitted
                        # after the NEXT group's score prologue so the lb
                        # matmul never heads the PE queue while the DVE
                        # reciprocal chain is still in flight
                        fh, fgsl, fot, fl, last = fin
                        linv_f = p2lb.tile([1, SG], f32, name="linv_f")
                        nc.vector.reciprocal_approx_fast(linv_f[:], fl[:])
                        linv_r = p2lb.tile([1, SG], f32r, name="linv_r")
                        nc.vector.tensor_copy(linv_r[:], linv_f[:])
                        lb_ps = p2opsum.tile([128, SG], f32, name="lb_ps",
                                             tag="lnorm", bufs=1)
                        nc.tensor.matmul(lb_ps[:], lhsT=onesr_sb[:],
                                         rhs=linv_r[:], start=True, stop=True)
                        lb_sb = p2lb.tile([128, SG], f32, name="lb_sb")
                        nc.vector.tensor_copy(lb_sb[:], lb_ps[:])
                        on_sb = p2tmp.tile([128, SG], bf16, name="on_sb")
                        nc.vector.tensor_tensor(on_sb[:], fot[:], lb_sb[:],
                                                mybir.AluOpType.mult)
                        nc.sync.dma_start(cc_in[fh][:, fgsl], on_sb[:])
                        if last:
                            nc.gpsimd.collective_compute(
                                "AllGather", mybir.AluOpType.bypass,
                                ins=[cc_in[fh].opt()], outs=[cc_out[fh].opt()],
                                replica_groups=[list(range(NCORES))],
                            )
                            # stage this head's gathered O into SBUF right
                            # away so the output projection can start the
                            # moment the PE queue drains phase 2. gpsimd
                            # queue: idle after startup, so the AG-wait here
                            # cannot stall the sync queue's qt_g prefetches
                            oh = ohpool.tile([128, NCORES, S], bf16, name="oh",
                                             tag="oh")
                            nc.gpsimd.dma_start(
                                oh[:],
                                cc_out[fh].rearrange("(r p) s -> p r s", p=128))
                            oh_tiles.append(oh)

                    pending_fin = None
                    for h in range(QH):
                        for g in range(G):
                            gsl = slice(g * SG, (g + 1) * SG)
                            qt_g = p2tmp.tile([128, SG], bf16, name="qt_g", bufs=4)
                            nc.sync.dma_start(
                                qt_g[:], qt_spill[h * 128:(h + 1) * 128, gsl])

                            nk = 4 * g + 4
                            DEPTH = 5
                            st_tiles = {}

                            def off_of(j):
                                # fully-masked q columns skipped on diagonal tiles
                                return max(0, (j - 4 * g)) * 128

                            def do_st(j, qt_g=qt_g, st_tiles=st_tiles,
                                      off_of=off_of):
                                off = off_of(j)
                                stp = p2psum.tile([128, SG], f32, name="st_ps",
                                                  tag="st_ps", bufs=DEPTH)
                                nc.tensor.matmul(
                                    stp[:, off:], lhsT=kt_sb[:, j * 128:(j + 1) * 128],
                                    rhs=qt_g[:, off:], start=True, stop=True)
                                if j >= 4 * g:
                                    # additive causal mask applied in PSUM,
                                    # ahead of the exp -> AV critical path
                                    nc.vector.tensor_tensor(
                                        stp[:, off:off + 128],
                                        stp[:, off:off + 128], madd_sb[:],
                                        mybir.AluOpType.add)
                                st_tiles[j] = stp

                            for j in range(min(DEPTH - 1, nk)):
                                do_st(j)
                            if pending_fin is not None:
                                finalize(pending_fin)

                            ot_ps = p2opsum.tile([128, SG], f32, name="ot_ps", bufs=2)
                            l_ps = p2opsum.tile([1, SG], f32, name="l_ps",
                                                tag="lnorm", bufs=1)

                            def do_av(j, put, ot_ps=ot_ps, l_ps=l_ps,
                                      off_of=off_of, nk=nk):
                                off = off_of(j)
                                nc.tensor.matmul(ot_ps[:, off:], lhsT=v_sb[:, j, :],
                                                 rhs=put[:, off:],
                                                 start=(j == 0), stop=(j == nk - 1),
                                                 skip_group_check=True)
                                nc.tensor.matmul(l_ps[:, off:], lhsT=onesc_sb[:],
                                                 rhs=put[:, off:],
                                                 start=(j == 0), stop=(j == nk - 1),
                                                 skip_group_check=True)

                            # AV/l consumption lags the exp by one j so the
                            # ACT completion semaphore is always set a full
                            # iteration before the PE reaches its consumer
                            prev_av = None
                            for j in range(nk):
                                off = off_of(j)
                                st_ps = st_tiles.pop(j)
                                put = p2tmp.tile([128, SG], bf16, name="put", bufs=6)
                                nc.scalar.activation(put[:, off:], st_ps[:, off:],
                                                     mybir.ActivationFunctionType.Exp,
                                                     scale=SCALE)
                                if prev_av is not None:
                                    do_av(*prev_av)
                                # lookahead DEPTH-1: the new score tile reuses
                                # the slot freed by the PREVIOUS ACT, which
                                # (like the AV above) already has a full
                                # iteration of semaphore margin
                                if j + DEPTH - 1 < nk:
                                    do_st(j + DEPTH - 1)
                                prev_av = (j, put)
                            do_av(*prev_av)

                            pending_fin = (h, gsl, ot_ps, l_ps, g == G - 1)
                    finalize(pending_fin)

                # ---------------- Phase 3: yT += wo_h^T @ O_h^T per head
                with (
                    tc.tile_pool(name="p3out", bufs=3) as p3out,
                    tc.tile_pool(name="p3psum", bufs=2, space="PSUM") as p3psum,
                ):
                    for h in range(QH):
                        oh = oh_tiles[h]
                        for dt in range(4):
                            y_ps = p3psum.tile([128, 4, SG], f32, name="y_ps",
                                               bufs=2)
                            for r in range(NCORES):
                                kt2 = h * NCORES + r
                                for sq in range(4):
                                    nc.tensor.matmul(
                                        y_ps[:, sq, :],
                                        lhsT=wo_sb[:, kt2,
                                                   dt * 128:(dt + 1) * 128],
                                        rhs=oh[:, r, sq * SG:(sq + 1) * SG],
                                        start=(r == 0), stop=(r == NCORES - 1))
                            for sq in range(4):
                                if h == 0:
                                    nc.vector.tensor_copy(
                                        y_acc[:, dt, sq, :], y_ps[:, sq, :])
                                elif h < QH - 1:
                                    nc.vector.tensor_tensor(
                                        y_acc[:, dt, sq, :],
                                        y_acc[:, dt, sq, :], y_ps[:, sq, :],
                                        mybir.AluOpType.add)
                                else:
                                    y_fin = p3out.tile([128, SG], f32,
                                                       name="y_fin")
                                    nc.vector.tensor_tensor(
                                        y_fin[:], y_acc[:, dt, sq, :],
                                        y_ps[:, sq, :], mybir.AluOpType.add)
                                    nc.sync.dma_start(
                                        yt[dt * 128:(dt + 1) * 128,
                                           sq * SG:(sq + 1) * SG], y_fin[:])
    nc.compile()
    return nc


def _host_prep(inputs):
    bf = ml_dtypes.bfloat16
    x = np.asarray(inputs["x"], dtype=np.float32)
    wq = np.asarray(inputs["wq"], dtype=np.float32)
    wk = np.asarray(inputs["wk"], dtype=np.float32)
    wv = np.asarray(inputs["wv"], dtype=np.float32)
    wo = np.asarray(inputs["wo"], dtype=np.float32)
    cos = np.asarray(inputs["freqs_cos"], dtype=np.float32)
    sin = np.asarray(inputs["freqs_sin"], dtype=np.float32)
    mask = np.asarray(inputs["mask"], dtype=np.float32)

    # xt[st, p, kt, s] = x[128*st + s, 128*kt + p]
    xts = np.ascontiguousarray(
        x.reshape(ST, 128, DK, 128).transpose(0, 3, 2, 1)).astype(bf)

    # de-interleave RoPE pairs within each head: evens then odds
    perm = np.concatenate([np.arange(0, HD, 2), np.arange(1, HD, 2)])

    cos_t = np.ascontiguousarray(cos.reshape(ST, 128, 64).transpose(1, 0, 2))
    sin_t = np.ascontiguousarray(sin.reshape(ST, 128, 64).transpose(1, 0, 2))

    # additive causal mask for a diagonal 128x128 tile [k-in-tile, q-in-tile],
    # derived from the actual mask input (g-independent for causal)
    madd_t = np.ascontiguousarray(
        np.where(mask[0:128, 0:128].T == 0.0, 0.0, -30000.0)).astype(np.float32)

    def ktile(w):  # [D, m] -> [128, DK, m]
        return np.ascontiguousarray(
            w.reshape(DK, 128, w.shape[1]).transpose(1, 0, 2))

    in_maps = []
    for c in range(NCORES):
        wq_c = wq[:, 512 * c:512 * (c + 1)].reshape(D, QH, HD)[:, :, perm]
        wq_c = wq_c.reshape(D, QH * HD)
        wk_c = wk[:, 128 * c:128 * (c + 1)][:, perm]
        wv_c = wv[:, 128 * c:128 * (c + 1)]
        wkv_c = np.concatenate([wk_c, wv_c], axis=1)
        # wo rows reordered to (head, rank, d) to match per-head AllGather
        wo_c = wo[:, 512 * c:512 * (c + 1)]
        wo_c = wo_c.reshape(NCORES, QH, 128, 512).transpose(1, 0, 2, 3)
        wo_c = wo_c.reshape(D, 512)
        in_maps.append({
            "xt": xts,
            "wq": ktile(wq_c).astype(bf),
            "wkv": ktile(wkv_c).astype(bf),
            "wo": ktile(wo_c).astype(bf),
            "cos": cos_t,
            "sin": sin_t,
            "madd": madd_t,
            "onesc": np.ones((128, 1), dtype=bf),
            "onesr": np.ones((1, 128), dtype=np.float32),
            "ident": np.eye(128, dtype=bf),
        })
    return in_maps


def _run(inputs, trace=False, tmpdir=None):
    from concourse import bass_utils
    if "nc" not in _CACHE:
        _CACHE["nc"] = _build()
    nc = _CACHE["nc"]
    in_maps = _host_prep(inputs)
    res = bass_utils.run_bass_kernel_spmd(
        nc, in_maps, core_ids=list(range(NCORES)), trace=trace, tmpdir=tmpdir)
    yts = [res.results[c]["yt"] for c in range(NCORES)]
    y = np.concatenate([t.T for t in yts], axis=1).astype(np.float32)
    return y.reshape(1, S, D), res


def kernel(**inputs):
    y, _ = _run(inputs, trace=False)
    return y


# revision 39
# speedup vs baseline: 1.0628x; 1.0066x over previous
"""GQA attention layer (B=1, S=2048, D=4096, H=32, KV=8, HD=128) on 8 TRN2
NeuronCores. Tensor-parallel over the 8 KV-head groups; per-head AllGather of
the attention outputs (overlapped with attention compute), then each core
computes one column shard of the output projection (no AllReduce needed).

Datapath is bf16 into fp32 PSUM accumulation (x/wq/wkv/q/k/v/P/O/wo in bf16;
scores, softmax sums and the output projection accumulate in fp32). Attention
is computed in transposed orientation (scores^T = kT-slices as lhsT against
qT) so softmax sums become TensorEngine matmuls and no P-transposes are
needed. Exp runs on paired score tiles (1024-wide) to amortize ACT overhead.
The output projection accumulates per-head partials into an SBUF fp32
accumulator so each head's matmuls start as soon as its AllGather lands.
RoPE pairs are de-interleaved by permuting wq/wk columns host-side; all DRAM
operands are host-pretiled for contiguous DMA.
"""
import numpy as np
import ml_dtypes

S = 2048
D = 4096
HD = 128
QH = 4            # q heads per core
NCORES = 8
ST = S // 128     # 16 s-tiles
DK = D // 128     # 32 contraction tiles
G = 4             # q groups per head
SG = S // G       # 512 q positions per group
SCALE = 1.0 / np.sqrt(128.0)

_CACHE = {}


def _build():
    import concourse.mybir as mybir
    import concourse.tile as tile
    from concourse import bacc

    f32, f32r = mybir.dt.float32, mybir.dt.float32r
    bf16 = mybir.dt.bfloat16
    nc = bacc.Bacc("TRN2", target_bir_lowering=False, debug=False,
                   num_devices=NCORES)

    xt = nc.dram_tensor("xt", [ST, 128, DK, 128], bf16, kind="ExternalInput").ap()
    wq = nc.dram_tensor("wq", [128, DK, QH * HD], bf16, kind="ExternalInput").ap()
    wkv = nc.dram_tensor("wkv", [128, DK, 2 * HD], bf16, kind="ExternalInput").ap()
    wo = nc.dram_tensor("wo", [128, DK, 512], bf16, kind="ExternalInput").ap()
    cos = nc.dram_tensor("cos", [128, ST, 64], f32, kind="ExternalInput").ap()
    sin = nc.dram_tensor("sin", [128, ST, 64], f32, kind="ExternalInput").ap()
    madd = nc.dram_tensor("madd", [128, 128], f32, kind="ExternalInput").ap()
    onesc = nc.dram_tensor("onesc", [128, 1], bf16, kind="ExternalInput").ap()
    onesr = nc.dram_tensor("onesr", [1, 128], f32r, kind="ExternalInput").ap()
    ident = nc.dram_tensor("ident", [128, 128], bf16, kind="ExternalInput").ap()
    yt = nc.dram_tensor("yt", [512, S], f32, kind="ExternalOutput").ap()

    with tile.TileContext(nc) as tc:
        with (
            tc.tile_pool(name="const", bufs=1) as constp,
            tc.tile_pool(name="resid", bufs=1) as resid,
            tc.tile_pool(name="dram", bufs=1, space="DRAM") as dram,
        ):
            cos_sb = constp.tile([128, ST, 64], f32)
            sin_sb = constp.tile([128, ST, 64], f32)
            madd_sb = constp.tile([128, 128], f32)
            onesc_sb = constp.tile([128, 1], bf16)
            onesr_sb = constp.tile([1, 128], f32r)
            ident_sb = constp.tile([128, 128], bf16)

            # residents across phases
            kt_sb = resid.tile([128, S], bf16)           # roped K^T  [d, s]
            v_sb = resid.tile([128, ST, HD], bf16)       # natural V  [s-in-tile, st, d]

            qt_spill = dram.tile([QH * 128, S], bf16)    # roped Q^T per head
            cc_in = [dram.tile([128, S], bf16, name=f"cc_in{h}") for h in range(QH)]
            cc_out = [dram.tile([NCORES * 128, S], bf16, addr_space="Shared",
                                name=f"cc_out{h}") for h in range(QH)]

            # ---------------- Phase 1: QKV projections + RoPE + transposes
            with (
                tc.tile_pool(name="wpool", bufs=1) as wpool,
                tc.tile_pool(name="xpool", bufs=3) as xpool,
                tc.tile_pool(name="p1tmp", bufs=3) as p1tmp,
                tc.tile_pool(name="p1psum", bufs=2, space="PSUM") as p1psum,
            ):
                # first projection-weight chunks at the head of the gpsimd
                # queue (ahead of the constants), in parallel with xt chunk 0
                # on the sync queue: first matmul gate ~2.5us of DMA, not ~7us
                wq_sb = wpool.tile([128, DK, QH * HD], bf16)
                wkv_sb = wpool.tile([128, DK, 2 * HD], bf16)
                nc.gpsimd.dma_start(wq_sb[:, 0:4, :], wq[:, 0:4, :])
                nc.gpsimd.dma_start(wkv_sb[:, 0:4, :], wkv[:, 0:4, :])
                nc.gpsimd.dma_start(cos_sb[:], cos)
                nc.gpsimd.dma_start(sin_sb[:], sin)
                nc.gpsimd.dma_start(madd_sb[:], madd)
                nc.gpsimd.dma_start(onesc_sb[:], onesc)
                nc.gpsimd.dma_start(onesr_sb[:], onesr)
                nc.gpsimd.dma_start(ident_sb[:], ident)

                def emit_transposes(qnat, knat, ssl):
                    # deferred one s-tile so PE never waits on the DVE rope
                    qts = p1tmp.tile([128, QH, 128], bf16, name="qts")
                    for h in range(QH):
                        qtr = p1psum.tile([128, 128], bf16, name="qtr")
                        nc.tensor.transpose(qtr[:], qnat[:, h, :], ident_sb[:])
                        nc.vector.tensor_copy(qts[:, h, :], qtr[:])
                    nc.sync.dma_start(
                        qt_spill.rearrange("(h p) s -> p h s", p=128)[:, :, ssl],
                        qts[:])
                    ktr = p1psum.tile([128, 128], bf16, name="ktr")
                    nc.tensor.transpose(ktr[:], knat[:], ident_sb[:])
                    nc.vector.tensor_copy(kt_sb[:, ssl], ktr[:])

                pending = None
                for st in range(ST):
                    ssl = slice(st * 128, (st + 1) * 128)
                    xt_st = xpool.tile([128, DK, 128], bf16, name="xt_st")
                    if st == 0:
                        # interleave x/weight chunks so the first matmuls
                        # unblock as soon as the leading chunks land
                        for c in range(8):
                            ksl = slice(c * 4, (c + 1) * 4)
                            nc.sync.dma_start(xt_st[:, ksl, :], xt[st][:, ksl, :])
                            if c >= 1:  # chunk 0 preloaded on gpsimd queue
                                nc.sync.dma_start(wq_sb[:, ksl, :], wq[:, ksl, :])
                                nc.sync.dma_start(wkv_sb[:, ksl, :], wkv[:, ksl, :])
                    else:
                        nc.sync.dma_start(xt_st[:], xt[st])

                    q_ps = p1psum.tile([128, QH * HD], f32, name="q_ps")
                    kv_ps = p1psum.tile([128, 2 * HD], f32, name="kv_ps")
                    for kt in range(DK):
                        nc.tensor.matmul(q_ps[:], lhsT=xt_st[:, kt], rhs=wq_sb[:, kt],
                                         start=(kt == 0), stop=(kt == DK - 1))
                        nc.tensor.matmul(kv_ps[:], lhsT=xt_st[:, kt], rhs=wkv_sb[:, kt],
                                         start=(kt == 0), stop=(kt == DK - 1))
                    if pending is not None:
                        emit_transposes(*pending)

                    # RoPE on q (4 heads batched) during PSUM eviction.
                    qp = q_ps[:].rearrange("p (h d) -> p h d", h=QH)
                    qa, qb = qp[:, :, 0:64], qp[:, :, 64:128]
                    cbc = cos_sb[:, st:st + 1, :].to_broadcast([128, QH, 64])
                    sbc = sin_sb[:, st:st + 1, :].to_broadcast([128, QH, 64])
                    t1 = p1tmp.tile([128, QH, 64], f32, name="t1")
                    t2 = p1tmp.tile([128, QH, 64], f32, name="t2")
                    qnat = p1tmp.tile([128, QH, HD], bf16, name="qnat")
                    na, nb = qnat[:, :, 0:64], qnat[:, :, 64:128]
                    nc.vector.tensor_tensor(t1[:], qa, sbc, mybir.AluOpType.mult)
                    nc.vector.tensor_tensor(t2[:], qb, sbc, mybir.AluOpType.mult)
                    nc.vector.tensor_tensor(na, qa, cbc, mybir.AluOpType.mult)
                    nc.vector.tensor_tensor(nb, qb, cbc, mybir.AluOpType.mult)
                    nc.vector.tensor_tensor(na, na, t2[:], mybir.AluOpType.subtract)
                    nc.vector.tensor_tensor(nb, nb, t1[:], mybir.AluOpType.add)

                    # RoPE on k
                    ka, kb = kv_ps[:, 0:64], kv_ps[:, 64:128]
                    cb1 = cos_sb[:, st, :]
                    sb1 = sin_sb[:, st, :]
                    kt1 = p1tmp.tile([128, 64], f32, name="kt1")
                    kt2 = p1tmp.tile([128, 64], f32, name="kt2")
                    knat = p1tmp.tile([128, HD], bf16, name="knat")
                    kna, knb = knat[:, 0:64], knat[:, 64:128]
                    nc.vector.tensor_tensor(kt1[:], ka, sb1, mybir.AluOpType.mult)
                    nc.vector.tensor_tensor(kt2[:], kb, sb1, mybir.AluOpType.mult)
                    nc.vector.tensor_tensor(kna, ka, cb1, mybir.AluOpType.mult)
                    nc.vector.tensor_tensor(knb, kb, cb1, mybir.AluOpType.mult)
                    nc.vector.tensor_tensor(kna, kna, kt2[:], mybir.AluOpType.subtract)
                    nc.vector.tensor_tensor(knb, knb, kt1[:], mybir.AluOpType.add)

                    # V natural, straight copy
                    nc.vector.tensor_copy(v_sb[:, st, :], kv_ps[:, HD:2 * HD])

                    pending = (qnat, knat, ssl)
                emit_transposes(*pending)

            # pools that live through phases 2 and 3
            with (
                tc.tile_pool(name="wopool", bufs=1) as wopool,
                tc.tile_pool(name="ohpool", bufs=2) as ohpool,
                tc.tile_pool(name="yaccp", bufs=1) as yaccp,
            ):
                wo_sb = wopool.tile([128, DK, 512], bf16)
                nc.gpsimd.dma_start(wo_sb[:], wo)
                y_acc = yaccp.tile([128, 4, 4, SG], f32)
                oh_tiles = []

                # ---------------- Phase 2: attention per (head, group)
                with (
                    tc.tile_pool(name="p2tmp", bufs=3) as p2tmp,
                    tc.tile_pool(name="p2lb", bufs=2) as p2lb,
                    tc.tile_pool(name="p2psum", bufs=2, space="PSUM") as p2psum,
                    tc.tile_pool(name="p2opsum", bufs=2, space="PSUM") as p2opsum,
                ):
                    def finalize(fin):
                        # normalization chain for a finished group, emitted
                        # after the NEXT group's score prologue so the lb
                        # matmul never heads the PE queue while the DVE
                        # reciprocal chain is still in flight
                        fh, fgsl, fot, fl, last = fin
                        linv_f = p2lb.tile([1, SG], f32, name="linv_f")
                        nc.vector.reciprocal_approx_fast(linv_f[:], fl[:])
                        linv_r = p2lb.tile([1, SG], f32r, name="linv_r")
                        nc.vector.tensor_copy(linv_r[:], linv_f[:])
                        lb_ps = p2opsum.tile([128, SG], f32, name="lb_ps",
                                             tag="lnorm", bufs=1)
                        nc.tensor.matmul(lb_ps[:], lhsT=onesr_sb[:],
                                         rhs=linv_r[:], start=True, stop=True)
                        lb_sb = p2lb.tile([128, SG], f32, name="lb_sb")
                        nc.vector.tensor_copy(lb_sb[:], lb_ps[:])
                        on_sb = p2tmp.tile([128, SG], bf16, name="on_sb")
                        nc.vector.tensor_tensor(on_sb[:], fot[:], lb_sb[:],
                                                mybir.AluOpType.mult)
                        nc.sync.dma_start(cc_in[fh][:, fgsl], on_sb[:])
                        if last:
                            nc.gpsimd.collective_compute(
                                "AllGather", mybir.AluOpType.bypass,
                                ins=[cc_in[fh].opt()], outs=[cc_out[fh].opt()],
                                replica_groups=[list(range(NCORES))],
                            )
                            # stage this head's gathered O into SBUF right
                            # away so the output projection can start the
                            # moment the PE queue drains phase 2. gpsimd
                            # queue: idle after startup, so the AG-wait here
                            # cannot stall the sync queue's qt_g prefetches
                            oh = ohpool.tile([128, NCORES, S], bf16, name="oh",
                                             tag="oh")
                            nc.gpsimd.dma_start(
                                oh[:],
                                cc_out[fh].rearrange("(r p) s -> p r s", p=128))
                            oh_tiles.append(oh)

                    pending_fin = None
                    for h in range(QH):
                        for g in range(G):
                            gsl = slice(g * SG, (g + 1) * SG)
                            qt_g = p2tmp.tile([128, SG], bf16, name="qt_g", bufs=4)
                            nc.sync.dma_start(
                                qt_g[:], qt_spill[h * 128:(h + 1) * 128, gsl])

                            nk = 4 * g + 4
                            DEPTH = 5
                            st_tiles = {}

                            def off_of(j):
                                # fully-masked q columns skipped on diagonal tiles
                                return max(0, (j - 4 * g)) * 128

                            def do_st(j, qt_g=qt_g, st_tiles=st_tiles,
                                      off_of=off_of):
                                off = off_of(j)
                                stp = p2psum.tile([128, SG], f32, name="st_ps",
                                                  tag="st_ps", bufs=DEPTH)
                                nc.tensor.matmul(
                                    stp[:, off:], lhsT=kt_sb[:, j * 128:(j + 1) * 128],
                                    rhs=qt_g[:, off:], start=True, stop=True)
                                if j >= 4 * g:
                                    # additive causal mask applied in PSUM,
                                    # ahead of the exp -> AV critical path
                                    nc.vector.tensor_tensor(
                                        stp[:, off:off + 128],
                                        stp[:, off:off + 128], madd_sb[:],
                                        mybir.AluOpType.add)
                                st_tiles[j] = stp

                            for j in range(min(DEPTH, nk)):
                                do_st(j)
                            if pending_fin is not None:
                                finalize(pending_fin)

                            ot_ps = p2opsum.tile([128, SG], f32, name="ot_ps", bufs=2)
                            l_ps = p2opsum.tile([1, SG], f32, name="l_ps",
                                                tag="lnorm", bufs=1)

                            def do_av(j, put, ot_ps=ot_ps, l_ps=l_ps,
                                      off_of=off_of, nk=nk):
                                off = off_of(j)
                                nc.tensor.matmul(ot_ps[:, off:], lhsT=v_sb[:, j, :],
                                                 rhs=put[:, off:],
                                                 start=(j == 0), stop=(j == nk - 1),
                                                 skip_group_check=True)
                                nc.tensor.matmul(l_ps[:, off:], lhsT=onesc_sb[:],
                                                 rhs=put[:, off:],
                                                 start=(j == 0), stop=(j == nk - 1),
                                                 skip_group_check=True)

                            # AV/l consumption lags the exp by one j so the
                            # ACT completion semaphore is always set a full
                            # iteration before the PE reaches its consumer
                            prev_av = None
                            for j in range(nk):
                                off = off_of(j)
                                st_ps = st_tiles.pop(j)
                                put = p2tmp.tile([128, SG], bf16, name="put", bufs=6)
                                nc.scalar.activation(put[:, off:], st_ps[:, off:],
                                                     mybir.ActivationFunctionType.Exp,
                                                     scale=SCALE)
                                if prev_av is not None:
                                    do_av(*prev_av)
                                if j + DEPTH < nk:
                                    do_st(j + DEPTH)
                                prev_av = (j, put)
                            do_av(*prev_av)

                            pending_fin = (h, gsl, ot_ps, l_ps, g == G - 1)
                    finalize(pending_fin)

                # ---------------- Phase 3: yT += wo_h^T @ O_h^T per head
                with (
                    tc.tile_pool(name="p3out", bufs=3) as p3out,
                    tc.tile_pool(name="p3psum", bufs=2, space="PSUM") as p3psum,
                ):
                    for h in range(QH):
                        oh = oh_tiles[h]
                        for dt in range(4):
                            y_ps = p3psum.tile([128, 4, SG], f32, name="y_ps",
                                               bufs=2)
                            for r in range(NCORES):
                                kt2 = h * NCORES + r
                                for sq in range(4):
                                    nc.tensor.matmul(
                                        y_ps[:, sq, :],
                                        lhsT=wo_sb[:, kt2,
                                                   dt * 128:(dt + 1) * 128],
                                        rhs=oh[:, r, sq * SG:(sq + 1) * SG],
                                        start=(r == 0), stop=(r == NCORES - 1))
                            for sq in range(4):
                                if h == 0:
                                    nc.vector.tensor_copy(
                                        y_acc[:, dt, sq, :], y_ps[:, sq, :])
                                elif h < QH - 1:
                                    nc.vector.tensor_tensor(
                                        y_acc[:, dt, sq, :],
                                        y_acc[:, dt, sq, :], y_ps[:, sq, :],
                                        mybir.AluOpType.add)
                                else:
                                    y_fin = p3out.tile([128, SG], f32,
                                                       name="y_fin")
                                    nc.vector.tensor_tensor(
                                        y_fin[:], y_acc[:, dt, sq, :],
                                        y_ps[:, sq, :], mybir.AluOpType.add)
                                    nc.sync.dma_start(
                                        yt[dt * 128:(dt + 1) * 128,
                                           sq * SG:(sq + 1) * SG], y_fin[:])
    nc.compile()
    return nc


def _host_prep(inputs):
    bf = ml_dtypes.bfloat16
    x = np.asarray(inputs["x"], dtype=np.float32)
    wq = np.asarray(inputs["wq"], dtype=np.float32)
    wk = np.asarray(inputs["wk"], dtype=np.float32)
    wv = np.asarray(inputs["wv"], dtype=np.float32)
    wo = np.asarray(inputs["wo"], dtype=np.float32)
    cos = np.asarray(inputs["freqs_cos"], dtype=np.float32)
    sin = np.asarray(inputs["freqs_sin"], dtype=np.float32)
    mask = np.asarray(inputs["mask"], dtype=np.float32)

    # xt[st, p, kt, s] = x[128*st + s, 128*kt + p]
    xts = np.ascontiguousarray(
        x.reshape(ST, 128, DK, 128).transpose(0, 3, 2, 1)).astype(bf)

    # de-interleave RoPE pairs within each head: evens then odds
    perm = np.concatenate([np.arange(0, HD, 2), np.arange(1, HD, 2)])

    cos_t = np.ascontiguousarray(cos.reshape(ST, 128, 64).transpose(1, 0, 2))
    sin_t = np.ascontiguousarray(sin.reshape(ST, 128, 64).transpose(1, 0, 2))

    # additive causal mask for a diagonal 128x128 tile [k-in-tile, q-in-tile],
    # derived from the actual mask input (g-independent for causal)
    madd_t = np.ascontiguousarray(
        np.where(mask[0:128, 0:128].T == 0.0, 0.0, -30000.0)).astype(np.float32)

    def ktile(w):  # [D, m] -> [128, DK, m]
        return np.ascontiguousarray(
            w.reshape(DK, 128, w.shape[1]).transpose(1, 0, 2))

    in_maps = []
    for c in range(NCORES):
        wq_c = wq[:, 512 * c:512 * (c + 1)].reshape(D, QH, HD)[:, :, perm]
        wq_c = wq_c.reshape(D, QH * HD)
        wk_c = wk[:, 128 * c:128 * (c + 1)][:, perm]
        wv_c = wv[:, 128 * c:128 * (c + 1)]
        wkv_c = np.concatenate([wk_c, wv_c], axis=1)
        # wo rows reordered to (head, rank, d) to match per-head AllGather
        wo_c = wo[:, 512 * c:512 * (c + 1)]
        wo_c = wo_c.reshape(NCORES, QH, 128, 512).transpose(1, 0, 2, 3)
        wo_c = wo_c.reshape(D, 512)
        in_maps.append({
            "xt": xts,
            "wq": ktile(wq_c).astype(bf),
            "wkv": ktile(wkv_c).astype(bf),
            "wo": ktile(wo_c).astype(bf),
            "cos": cos_t,
            "sin": sin_t,
            "madd": madd_t,
            "onesc": np.ones((128, 1), dtype=bf),
            "onesr": np.ones((1, 128), dtype=np.float32),
            "ident": np.eye(128, dtype=bf),
        })
    return in_maps


def _run(inputs, trace=False, tmpdir=None):
    from concourse import bass_utils
    if "nc" not in _CACHE:
        _CACHE["nc"] = _build()
    nc = _CACHE["nc"]
    in_maps = _host_prep(inputs)
    res = bass_utils.run_bass_kernel_spmd(
        nc, in_maps, core_ids=list(range(NCORES)), trace=trace, tmpdir=tmpdir)
    yts = [res.results[c]["yt"] for c in range(NCORES)]
    y = np.concatenate([t.T for t in yts], axis=1).astype(np.float32)
    return y.reshape(1, S, D), res


def kernel(**inputs):
    y, _ = _run(inputs, trace=False)
    return y
